# revision 1
# baseline (speedup 1.0000x reference)
"""Trainium2 Bass kernel for a 2-layer GraphSAGE (LSTM aggregator) GNN encoder.

Math (matches the fp32 jax reference):
  L1: h1 = relu(feat @ Wself1 + LSTM16(feat[nbr]) @ Wneigh1 + b1)
  L2: h2 = h1 @ Wself2 + LSTM16(h1[nbr]) @ Wneigh2 + b2
  pool: x[g] = mean_{node in graph g} h2 ; heads: (x@Wmu+bmu, x@Wsig+bsig)

Distribution: nodes sharded across 8 cores (4096 each). Small weights
replicated. h1 is all-gathered (bf16) between layers so every core can gather
arbitrary neighbor rows. Per-graph sums are computed per-core against global
graph ids and all-reduced; head matmuls run redundantly on every core.

On-core layout: the LSTM runs feature-major (gates^T = W @ X^T), with the
gathered neighbor features delivered directly in feature-major layout by
dma_gather(transpose=True) from bf16 tables in DRAM. LSTM state h/c stays
fp32; the ih-term matmuls are bf16 (inputs are bf16-rounded activations), the
hh-term matmuls are fp32.
"""

import numpy as np
import ml_dtypes

BF = ml_dtypes.bfloat16
F32 = np.float32

# full problem config
FULL = dict(N=32768, DEG=16, G=64, NCORE=8)
D_IN, D_FEAT, D_REP = 128, 256, 128


def build_program(N, DEG, G, NCORE, stop_after="full"):
    """Build + compile the SPMD Bass program. Returns the Bacc object.

    stop_after: "l1" = layer1 + allgather + h1 readback only (debug);
                "full" = whole network.
    """
    from contextlib import ExitStack

    import concourse.mybir as mybir
    import concourse.tile as tile
    from concourse import bacc, library_config

    f32 = mybir.dt.float32
    bf16 = mybir.dt.bfloat16
    i16 = mybir.dt.int16
    Sig = mybir.ActivationFunctionType.Sigmoid
    Tnh = mybir.ActivationFunctionType.Tanh
    Rlu = mybir.ActivationFunctionType.Relu

    NLOC = N // NCORE
    assert NLOC % 128 == 0
    L1G = 1024 if NLOC % 1024 == 0 else 512  # L1 node-group size
    L2G = 512                                # L2 node-group size
    NB = NLOC // 128                         # 128-node blocks
    shared = "Shared" if NCORE > 4 else "Local"

    nc = bacc.Bacc("TRN2", target_bir_lowering=False, debug=False,
                   num_devices=NCORE)

    # ---- DRAM I/O ----
    tab_feat = nc.dram_tensor("tab_feat", [N, D_IN], bf16, kind="ExternalInput")
    idx_steps = nc.dram_tensor("idx_steps", [128, DEG, NLOC // 16], i16,
                               kind="ExternalInput")
    idx_local = nc.dram_tensor("idx_local", [128, NLOC // 16], i16,
                               kind="ExternalInput")
    wihT1 = nc.dram_tensor("wihT1", [128, 4 * D_IN], bf16, kind="ExternalInput")
    whhT1 = nc.dram_tensor("whhT1", [128, 4 * D_IN], f32, kind="ExternalInput")
    blstm1 = nc.dram_tensor("blstm1", [128, 4], f32, kind="ExternalInput")
    wself1 = nc.dram_tensor("wself1", [128, D_FEAT], bf16, kind="ExternalInput")
    wneigh1 = nc.dram_tensor("wneigh1", [128, D_FEAT], f32, kind="ExternalInput")
    b1bc = nc.dram_tensor("b1bc", [128, D_FEAT], f32, kind="ExternalInput")
    wihT2 = nc.dram_tensor("wihT2", [128, 2, 4 * D_FEAT], bf16, kind="ExternalInput")
    whhT2 = nc.dram_tensor("whhT2", [128, 2, 4 * D_FEAT], f32, kind="ExternalInput")
    blstm2 = nc.dram_tensor("blstm2", [128, 8], f32, kind="ExternalInput")
    wself2 = nc.dram_tensor("wself2", [128, 2, D_FEAT], bf16, kind="ExternalInput")
    wneigh2 = nc.dram_tensor("wneigh2", [128, 2, D_FEAT], f32, kind="ExternalInput")
    b2bc = nc.dram_tensor("b2bc", [128, D_FEAT], f32, kind="ExternalInput")
    wmu = nc.dram_tensor("wmu", [128, 2, D_REP], f32, kind="ExternalInput")
    bmu = nc.dram_tensor("bmu", [G, D_REP], f32, kind="ExternalInput")
    wsig = nc.dram_tensor("wsig", [128, 2, D_REP], f32, kind="ExternalInput")
    bsig = nc.dram_tensor("bsig", [G, D_REP], f32, kind="ExternalInput")
    poolA = nc.dram_tensor("poolA", [128, NB, G], f32, kind="ExternalInput")

    if stop_after == "full":
        out_mu = nc.dram_tensor("out_mu", [G, D_REP], f32, kind="ExternalOutput")
        out_sigma = nc.dram_tensor("out_sigma", [G, D_REP], f32,
                                   kind="ExternalOutput")
    else:
        dbg_h1 = nc.dram_tensor("dbg_h1", [N, D_FEAT], bf16,
                                kind="ExternalOutput")

    h1_shard = nc.dram_tensor("h1_shard", [NLOC, D_FEAT], bf16, kind="Internal")
    h1_full = nc.dram_tensor("h1_full", [N, D_FEAT], bf16, kind="Internal",
                             addr_space=shared)
    pr_in = nc.dram_tensor("pr_in", [128, 2, G], f32, kind="Internal")
    pr_out = nc.dram_tensor("pr_out", [128, 2, G], f32, kind="Internal",
                            addr_space=shared)

    nc.gpsimd.load_library(library_config.mlp)

    with tile.TileContext(nc) as tc, ExitStack() as ctx:
        consts = ctx.enter_context(tc.tile_pool(name="consts", bufs=1))

        def cload(dram, shape, dtype, tag):
            t = consts.tile(shape, dtype, tag=tag)
            nc.sync.dma_start(out=t, in_=dram[tuple(slice(None) for _ in shape)])
            return t

        idxs_sb = cload(idx_steps, [128, DEG, NLOC // 16], i16, "idxs")
        idxl_sb = cload(idx_local, [128, NLOC // 16], i16, "idxl")
        wihT1_sb = cload(wihT1, [128, 4 * D_IN], bf16, "wihT1")
        whhT1_sb = cload(whhT1, [128, 4 * D_IN], f32, "whhT1")
        blstm1_sb = cload(blstm1, [128, 4], f32, "blstm1")
        wself1_sb = cload(wself1, [128, D_FEAT], bf16, "wself1")
        wneigh1_sb = cload(wneigh1, [128, D_FEAT], f32, "wneigh1")
        b1bc_sb = cload(b1bc, [128, D_FEAT], f32, "b1bc")
        wihT2_sb = cload(wihT2, [128, 2, 4 * D_FEAT], bf16, "wihT2")
        whhT2_sb = cload(whhT2, [128, 2, 4 * D_FEAT], f32, "whhT2")
        blstm2_sb = cload(blstm2, [128, 8], f32, "blstm2")
        wself2_sb = cload(wself2, [128, 2, D_FEAT], bf16, "wself2")
        wneigh2_sb = cload(wneigh2, [128, 2, D_FEAT], f32, "wneigh2")
        b2bc_sb = cload(b2bc, [128, D_FEAT], f32, "b2bc")
        wmu_sb = cload(wmu, [128, 2, D_REP], f32, "wmu")
        bmu_sb = cload(bmu, [G, D_REP], f32, "bmu")
        wsig_sb = cload(wsig, [128, 2, D_REP], f32, "wsig")
        bsig_sb = cload(bsig, [G, D_REP], f32, "bsig")
        poolA_sb = cload(poolA, [128, NB, G], f32, "poolA")

        gts = ctx.enter_context(tc.tile_pool(name="gts", bufs=2))
        xgp = ctx.enter_context(tc.tile_pool(name="xgp", bufs=4))
        snp = ctx.enter_context(tc.tile_pool(name="snp", bufs=3))

        GATES = [("i", Sig), ("f", Sig), ("g", Tnh), ("o", Sig)]

        # ================= Layer 1 =================
        # Per node-group: LSTM -> self/neigh -> AllGather of that chunk, so
        # each chunk's collective overlaps the next group's LSTM compute.
        # h1_full is chunk-major ([chunk][rank][j]); the host permutes the
        # feature table and every gather index to match.
        with tc.tile_pool(name="st1", bufs=1) as st1:
            hN1 = st1.tile([128, NLOC], f32, tag="hN1")
            cN1 = st1.tile([128, NLOC], f32, tag="cN1")
            nc.vector.memset(hN1, 0.0)
            nc.vector.memset(cN1, 0.0)
            featT = st1.tile([128, 1, NLOC], bf16, tag="featT")
            nc.gpsimd.dma_gather(featT[:], tab_feat[:], idxl_sb[:, :],
                                 NLOC, NLOC, D_IN, transpose=True,
                                 single_packet=False)

            with tc.tile_pool(name="psl1", bufs=3, space="PSUM") as psl, \
                 tc.tile_pool(name="psm1", bufs=2, space="PSUM") as psm:
                for g in range(NLOC // L1G):
                    gsl = slice(g * L1G, (g + 1) * L1G)
                    isl = slice(g * (L1G // 16), (g + 1) * (L1G // 16))
                    for t in range(DEG):
                        xg = xgp.tile([128, 1, L1G], bf16, tag="xg1")
                        nc.gpsimd.dma_gather(
                            xg[:], tab_feat[:], idxs_sb[:, t, isl],
                            L1G, L1G, D_IN, transpose=True,
                            single_packet=False)
                        gate_sb = {}
                        for gi, (gn, func) in enumerate(GATES):
                            ps = psl.tile([128, L1G], f32, tag="ps1")
                            wsl = slice(gi * 128, (gi + 1) * 128)
                            for nh in range(L1G // 512):
                                o = ps[:, nh * 512:(nh + 1) * 512]
                                nc.tensor.matmul(
                                    o, wihT1_sb[:, wsl],
                                    xg[:, 0, nh * 512:(nh + 1) * 512],
                                    start=True, stop=False)
                                nc.tensor.matmul(
                                    o, whhT1_sb[:, wsl],
                                    hN1[:, g * L1G + nh * 512:g * L1G + (nh + 1) * 512],
                                    start=False, stop=True)
                            gt = gts.tile([128, L1G], f32, tag=f"gt{gn}")
                            nc.scalar.activation(gt, ps[:, :], func,
                                                 bias=blstm1_sb[:, gi:gi + 1])
                            gate_sb[gn] = gt
                        t0 = gts.tile([128, L1G], f32, tag="t0")
                        nc.vector.tensor_mul(t0, gate_sb["i"], gate_sb["g"])
                        nc.vector.tensor_mul(cN1[:, gsl], cN1[:, gsl], gate_sb["f"])
                        nc.vector.tensor_add(cN1[:, gsl], cN1[:, gsl], t0)
                        tch = gts.tile([128, L1G], f32, tag="tch")
                        nc.scalar.activation(tch, cN1[:, gsl], Tnh)
                        nc.vector.tensor_mul(hN1[:, gsl], gate_sb["o"], tch)

                    # self/neigh + relu for this group's blocks -> h1_shard
                    for blk in range(g * (L1G // 128), (g + 1) * (L1G // 128)):
                        bsl = slice(blk * 128, (blk + 1) * 128)
                        ps = psm.tile([128, D_FEAT], f32, tag="psm1")
                        nc.tensor.matmul(ps, featT[:, 0, bsl], wself1_sb[:, :],
                                         start=True, stop=False)
                        nc.tensor.matmul(ps, hN1[:, bsl], wneigh1_sb[:, :],
                                         start=False, stop=True)
                        tmp = snp.tile([128, D_FEAT], f32, tag="sn1t")
                        nc.vector.tensor_add(tmp, ps, b1bc_sb)
                        h1b = snp.tile([128, D_FEAT], bf16, tag="sn1b")
                        nc.scalar.activation(h1b, tmp, Rlu)
                        nc.sync.dma_start(out=h1_shard[bsl, :], in_=h1b)
                    # all-gather this chunk; overlaps the next group's LSTM
                    nc.gpsimd.collective_compute(
                        "AllGather", mybir.AluOpType.bypass,
                        replica_groups=[list(range(NCORE))],
                        ins=[h1_shard[g * L1G:(g + 1) * L1G, :]],
                        outs=[h1_full[g * NCORE * L1G:
                                      (g + 1) * NCORE * L1G, :]])

        import concourse.mybir as _mb

        if stop_after == "l1":
            with tc.tile_pool(name="dbgp", bufs=2) as dbgp:
                for blk in range(N // 128):
                    dt_ = dbgp.tile([128, D_FEAT], bf16, tag="dbg")
                    nc.sync.dma_start(
                        out=dt_, in_=h1_full[blk * 128:(blk + 1) * 128, :])
                    nc.sync.dma_start(
                        out=dbg_h1[blk * 128:(blk + 1) * 128, :], in_=dt_)
        else:
            # ================= Layer 2 =================
            with tc.tile_pool(name="st2", bufs=1) as st2:
                hN2 = st2.tile([128, 2, NLOC], f32, tag="hN2")
                cN2 = st2.tile([128, 2, NLOC], f32, tag="cN2")
                nc.vector.memset(hN2, 0.0)
                nc.vector.memset(cN2, 0.0)

                with tc.tile_pool(name="psl2", bufs=4, space="PSUM") as psl:
                    for g in range(NLOC // L2G):
                        gsl = slice(g * L2G, (g + 1) * L2G)
                        isl = slice(g * (L2G // 16), (g + 1) * (L2G // 16))
                        for t in range(DEG):
                            xg = xgp.tile([128, 2, L2G], bf16, tag="xg2")
                            nc.gpsimd.dma_gather(
                                xg[:], h1_full[:], idxs_sb[:, t, isl],
                                L2G, L2G, D_FEAT, transpose=True,
                                single_packet=False)
                            gate_sb = {}
                            for gi, (gn, func) in enumerate(GATES):
                                ps = psl.tile([128, 2, L2G], f32, tag="ps2")
                                gt = gts.tile([128, 2, L2G], f32, tag=f"gt{gn}")
                                for mb in range(2):
                                    o = ps[:, mb, :]
                                    wsl = slice(gi * 2 * D_FEAT // 2 + mb * 128,
                                                gi * 2 * D_FEAT // 2 + (mb + 1) * 128)
                                    for kb in range(2):
                                        nc.tensor.matmul(
                                            o, wihT2_sb[:, kb, wsl], xg[:, kb, :],
                                            start=(kb == 0), stop=False)
                                    for kb in range(2):
                                        nc.tensor.matmul(
                                            o, whhT2_sb[:, kb, wsl],
                                            hN2[:, kb, gsl],
                                            start=False, stop=(kb == 1))
                                    nc.scalar.activation(
                                        gt[:, mb, :], o, func,
                                        bias=blstm2_sb[:, 2 * gi + mb:2 * gi + mb + 1])
                                gate_sb[gn] = gt
                            t0 = gts.tile([128, 2, L2G], f32, tag="t0")
                            nc.vector.tensor_mul(t0, gate_sb["i"], gate_sb["g"])
                            nc.vector.tensor_mul(cN2[:, :, gsl], cN2[:, :, gsl],
                                                 gate_sb["f"])
                            nc.vector.tensor_add(cN2[:, :, gsl], cN2[:, :, gsl], t0)
                            tch = gts.tile([128, 2, L2G], f32, tag="tch")
                            nc.scalar.activation(tch, cN2[:, :, gsl], Tnh)
                            nc.vector.tensor_mul(hN2[:, :, gsl], gate_sb["o"], tch)

                # L2 self/neigh + pooling
                h1T = st2.tile([128, 2, NLOC], bf16, tag="h1T")
                nc.gpsimd.dma_gather(h1T[:], h1_full[:], idxl_sb[:, :],
                                     NLOC, NLOC, D_FEAT, transpose=True,
                                     single_packet=False)
                with tc.tile_pool(name="psm2", bufs=2, space="PSUM") as psm, \
                     tc.tile_pool(name="pspool", bufs=2, space="PSUM") as psp, \
                     tc.tile_pool(name="pshead", bufs=2, space="PSUM") as psh:
                    pool_ps = [psp.tile([128, G], f32, tag=f"pool{mh}",
                                        name=f"pool_ps{mh}")
                               for mh in range(2)]
                    for blk in range(NB):
                        bsl = slice(blk * 128, (blk + 1) * 128)
                        ps = psm.tile([128, D_FEAT], f32, tag="psm2")
                        for kb in range(2):
                            nc.tensor.matmul(ps, h1T[:, kb, bsl], wself2_sb[:, kb, :],
                                             start=(kb == 0), stop=False)
                        for kb in range(2):
                            nc.tensor.matmul(ps, hN2[:, kb, bsl], wneigh2_sb[:, kb, :],
                                             start=False, stop=(kb == 1))
                        h2sb = snp.tile([128, D_FEAT], f32, tag="h2sb")
                        nc.vector.tensor_add(h2sb, ps, b2bc_sb)
                        for mh in range(2):
                            nc.tensor.matmul(
                                pool_ps[mh], h2sb[:, mh * 128:(mh + 1) * 128],
                                poolA_sb[:, blk, :],
                                start=(blk == 0), stop=(blk == NB - 1),
                                skip_group_check=True)
                    prcp = snp.tile([128, 2, G], f32, tag="prcp")
                    for mh in range(2):
                        nc.vector.tensor_copy(prcp[:, mh, :], pool_ps[mh])
                    nc.sync.dma_start(out=pr_in[:, :, :], in_=prcp)
                    nc.gpsimd.collective_compute(
                        "AllReduce", _mb.AluOpType.add,
                        replica_groups=[list(range(NCORE))],
                        ins=[pr_in[:]], outs=[pr_out[:]])
                    prx = snp.tile([128, 2, G], f32, tag="prx")
                    nc.sync.dma_start(out=prx, in_=pr_out[:, :, :])
                    for wsb, bsb, osb in ((wmu_sb, bmu_sb, out_mu),
                                          (wsig_sb, bsig_sb, out_sigma)):
                        ph = psh.tile([G, D_REP], f32, tag="ph")
                        for kb in range(2):
                            nc.tensor.matmul(ph, prx[:, kb, :], wsb[:, kb, :],
                                             start=(kb == 0), stop=(kb == 1))
                        ores = snp.tile([G, D_REP], f32, tag="ores")
                        nc.vector.tensor_add(ores, ph, bsb)
                        nc.sync.dma_start(out=osb[:, :], in_=ores)


    nc.compile()
    return nc


def make_inmaps(inputs, N, DEG, G, NCORE):
    """Host-side preprocessing: shard + reformat the full inputs per core."""
    NLOC = N // NCORE
    NB = NLOC // 128

    feat = np.asarray(inputs["in_feat"], dtype=F32)
    nbr = np.asarray(inputs["neighbors"], dtype=np.int64)
    n2g = np.asarray(inputs["node2graph"], dtype=np.int64)

    def A(name):
        return np.asarray(inputs[name], dtype=F32)

    # chunk-major row permutation matching the on-device chunked AllGather:
    # node (rank r, chunk c, offset j) lives at table row c*(NCORE*L1G)+r*L1G+j
    L1G = 1024 if NLOC % 1024 == 0 else 512
    nodes = np.arange(N)
    r_, rem = nodes // NLOC, nodes % NLOC
    P = (rem // L1G) * (NCORE * L1G) + r_ * L1G + (rem % L1G)

    tab_feat = np.empty((N, feat.shape[1]), BF)
    tab_feat[P] = feat.astype(BF)
    nbr = P[nbr]

    def wrap_idx(ids):
        # ids [n] -> [128, n//16] int16, wrapped in 16 partitions and
        # replicated across the 8 gpsimd cores' partition stripes.
        n = ids.shape[0]
        w = ids.reshape(n // 16, 16).T.astype(np.int16)  # [16, n/16]
        return np.tile(w, (8, 1))

    wihT1 = np.ascontiguousarray(A("w_ih1").T).astype(BF)          # [128, 512]
    whhT1 = np.ascontiguousarray(A("w_hh1").T)                     # [128, 512]
    blstm1 = np.ascontiguousarray(A("b_lstm1").reshape(4, 128).T)  # [128, 4]
    wself1 = A("w_self1").astype(BF)                               # [128, 256]
    wneigh1 = A("w_neigh1")                                        # [128, 256]
    b1bc = np.tile(A("b1")[None, :], (128, 1))
    wihT2 = np.ascontiguousarray(
        A("w_ih2").T.reshape(2, 128, 4 * D_FEAT).transpose(1, 0, 2)).astype(BF)
    whhT2 = np.ascontiguousarray(
        A("w_hh2").T.reshape(2, 128, 4 * D_FEAT).transpose(1, 0, 2))
    blstm2 = np.ascontiguousarray(
        A("b_lstm2").reshape(4, 2, 128).transpose(2, 0, 1).reshape(128, 8))
    wself2 = np.ascontiguousarray(
        A("w_self2").reshape(2, 128, D_FEAT).transpose(1, 0, 2)).astype(BF)
    wneigh2 = np.ascontiguousarray(
        A("w_neigh2").reshape(2, 128, D_FEAT).transpose(1, 0, 2))
    b2bc = np.tile(A("b2")[None, :], (128, 1))
    wmu = np.ascontiguousarray(
        A("w_mu").reshape(2, 128, D_REP).transpose(1, 0, 2))
    bmu = np.tile(A("b_mu")[None, :], (G, 1))
    wsig = np.ascontiguousarray(
        A("w_sigma").reshape(2, 128, D_REP).transpose(1, 0, 2))
    bsig = np.tile(A("b_sigma")[None, :], (G, 1))

    cnt = np.bincount(n2g, minlength=G).astype(F32)
    inv = 1.0 / np.maximum(cnt, 1.0)

    common = dict(
        tab_feat=tab_feat, wihT1=wihT1, whhT1=whhT1.astype(F32),
        blstm1=blstm1.astype(F32), wself1=wself1,
        wneigh1=wneigh1.astype(F32), b1bc=b1bc.astype(F32),
        wihT2=wihT2, whhT2=whhT2.astype(F32), blstm2=blstm2.astype(F32),
        wself2=wself2, wneigh2=wneigh2.astype(F32), b2bc=b2bc.astype(F32),
        wmu=wmu.astype(F32), bmu=bmu.astype(F32),
        wsig=wsig.astype(F32), bsig=bsig.astype(F32),
    )

    in_maps = []
    for c in range(NCORE):
        base = c * NLOC
        ids = nbr[base:base + NLOC, :]  # [NLOC, DEG]
        idx_steps = np.zeros((128, DEG, NLOC // 16), np.int16)
        for t in range(DEG):
            idx_steps[:, t, :] = wrap_idx(ids[:, t])
        idx_local = wrap_idx(P[np.arange(base, base + NLOC)])
        pA = np.zeros((128, NB, G), F32)
        gl = n2g[base:base + NLOC].reshape(NB, 128)  # [blk, j]
        for blk in range(NB):
            pA[np.arange(128), blk, gl[blk]] = inv[gl[blk]]
        m = dict(common)
        m["idx_steps"] = idx_steps
        m["idx_local"] = idx_local
        m["poolA"] = pA
        in_maps.append(m)
    return in_maps


_PROG = None


def kernel(**inputs):
    global _PROG
    from concourse.bass_utils import run_bass_kernel_spmd

    cfg = FULL
    if _PROG is None:
        _PROG = build_program(**cfg)
    in_maps = make_inmaps(inputs, **cfg)
    res = run_bass_kernel_spmd(_PROG, in_maps, core_ids=list(range(cfg["NCORE"])))
    mu = np.asarray(res.results[0]["out_mu"], dtype=np.float32)
    sigma = np.asarray(res.results[0]["out_sigma"], dtype=np.float32)
    return (mu, sigma)



# revision 7
# speedup vs baseline: 6.9673x; 6.9673x over previous
"""Trainium2 Bass kernel for a 2-layer GraphSAGE (LSTM aggregator) GNN encoder.

Math (matches the fp32 jax reference):
  L1: h1 = relu(feat @ Wself1 + LSTM16(feat[nbr]) @ Wneigh1 + b1)
  L2: h2 = h1 @ Wself2 + LSTM16(h1[nbr]) @ Wneigh2 + b2
  pool: x[g] = mean_{node in graph g} h2 ; heads: (x@Wmu+bmu, x@Wsig+bsig)

Distribution: nodes sharded across 8 cores (4096 each). The host->device
link (axon tunnel) is very slow, so per-call upload is minimized:
  - feature table: each core uploads only its 1MB shard; the full
    (chunk-major) table is reassembled on device with chunked AllGathers.
  - weights: packed into one f32 and one bf16 blob, uploaded as 1/8
    partition-shards and AllGathered on device.
  - gather indices: uploaded once per core at 1/8 partition height and
    replicated to 128 partitions on device.
  - pooling matrix: built on device from per-node graph ids + inverse
    counts with a fused is_equal/mult tensor_scalar against an iota row.

On-core layout: the LSTM runs feature-major (gates^T = W @ X^T), with the
gathered neighbor features delivered directly in feature-major layout by
dma_gather(transpose=True) from bf16 tables in DRAM. LSTM state h/c stays
fp32; the ih-term matmuls are bf16 (inputs are bf16-rounded activations), the
hh-term matmuls are fp32. Per-graph sums are computed per-core against global
graph ids and all-reduced; head matmuls run redundantly on every core.
"""

import numpy as np
import ml_dtypes

BF = ml_dtypes.bfloat16
F32 = np.float32

# full problem config
FULL = dict(N=32768, DEG=16, G=64, NCORE=8)
D_IN, D_FEAT, D_REP = 128, 256, 128


def _f32_layout():
    segs = [("whhT1", 512), ("wneigh1", 256), ("b1bc", 256), ("whhT2", 2048),
            ("wneigh2", 512), ("b2bc", 256), ("wmu", 256), ("wsig", 256),
            ("blstm1", 4), ("blstm2", 8), ("iota", 64), ("bmu", 128),
            ("bsig", 128)]
    off, o = {}, 0
    for n, w in segs:
        off[n] = (o, w)
        o += w
    o = (o + 15) // 16 * 16
    return off, o


def _bf_layout():
    segs = [("wihT1", 512), ("wself1", 256), ("wihT2", 2048), ("wself2", 512)]
    off, o = {}, 0
    for n, w in segs:
        off[n] = (o, w)
        o += w
    o = (o + 15) // 16 * 16
    return off, o


def build_program(N, DEG, G, NCORE, stop_after="full"):
    """Build + compile the SPMD Bass program. Returns the Bacc object."""
    from contextlib import ExitStack

    import concourse.mybir as mybir
    import concourse.tile as tile
    from concourse import bacc, library_config

    f32 = mybir.dt.float32
    bf16 = mybir.dt.bfloat16
    i16 = mybir.dt.int16
    Sig = mybir.ActivationFunctionType.Sigmoid
    Tnh = mybir.ActivationFunctionType.Tanh
    Rlu = mybir.ActivationFunctionType.Relu

    NLOC = N // NCORE
    assert NLOC % 128 == 0
    L1G = 1024 if NLOC % 1024 == 0 else 512  # L1 node-group size
    L2G = 512                                # L2 node-group size
    NB = NLOC // 128                         # 128-node blocks
    shared = "Shared" if NCORE > 4 else "Local"
    grp = [list(range(NCORE))]

    FOFF, F32C = _f32_layout()
    BOFF, BFC = _bf_layout()

    nc = bacc.Bacc("TRN2", target_bir_lowering=False, debug=False,
                   num_devices=NCORE)

    # ---- DRAM I/O (per-core, minimized for the slow host link) ----
    tab_shard = nc.dram_tensor("tab_shard", [NLOC, D_IN], bf16,
                               kind="ExternalInput")
    # [16, DEG+1, NLOC//16]: slots 0..DEG-1 = neighbor gather indices into the
    # chunk-major full table; slot DEG = local arange (for featT/h1T gathers).
    idxs_all = nc.dram_tensor("idxs_all", [16, DEG + 1, NLOC // 16], i16,
                              kind="ExternalInput")
    # [128, 2*NB]: cols 0:NB per-node graph id, NB:2*NB inverse graph size
    poolmeta = nc.dram_tensor("poolmeta", [128, 2 * NB], f32,
                              kind="ExternalInput")
    wf32_sh = nc.dram_tensor("wf32_sh", [128 // NCORE, F32C], f32,
                             kind="ExternalInput")
    wbf_sh = nc.dram_tensor("wbf_sh", [128 // NCORE, BFC], bf16,
                            kind="ExternalInput")

    out_mu = nc.dram_tensor("out_mu", [G, D_REP], f32, kind="ExternalOutput")
    out_sigma = nc.dram_tensor("out_sigma", [G, D_REP], f32,
                               kind="ExternalOutput")

    # ---- Internal DRAM ----
    # collectives may not read ExternalInput tensors; stage through these
    tab_loc = nc.dram_tensor("tab_loc", [NLOC, D_IN], bf16, kind="Internal")
    wf32_loc = nc.dram_tensor("wf32_loc", [128 // NCORE, F32C], f32,
                              kind="Internal")
    wbf_loc = nc.dram_tensor("wbf_loc", [128 // NCORE, BFC], bf16,
                             kind="Internal")
    tab_full = nc.dram_tensor("tab_full", [N, D_IN], bf16, kind="Internal",
                              addr_space=shared)
    wf32 = nc.dram_tensor("wf32", [128, F32C], f32, kind="Internal",
                          addr_space=shared)
    wbf = nc.dram_tensor("wbf", [128, BFC], bf16, kind="Internal",
                         addr_space=shared)
    h1_shard = nc.dram_tensor("h1_shard", [NLOC, D_FEAT], bf16, kind="Internal")
    h1_full = nc.dram_tensor("h1_full", [N, D_FEAT], bf16, kind="Internal",
                             addr_space=shared)
    pr_in = nc.dram_tensor("pr_in", [128, 2, G], f32, kind="Internal")
    pr_out = nc.dram_tensor("pr_out", [128, 2, G], f32, kind="Internal",
                            addr_space=shared)

    nc.gpsimd.load_library(library_config.mlp)

    with tile.TileContext(nc) as tc, ExitStack() as ctx:
        # stage ExternalInputs into Internal DRAM via SBUF (collectives may
        # not read IO tensors directly)
        with tc.tile_pool(name="stage", bufs=1) as stgp:
            stg_f = stgp.tile([128 // NCORE, F32C], f32, tag="stg_f")
            nc.sync.dma_start(out=stg_f, in_=wf32_sh[:, :])
            nc.sync.dma_start(out=wf32_loc[:, :], in_=stg_f)
            stg_b = stgp.tile([128 // NCORE, BFC], bf16, tag="stg_b")
            nc.sync.dma_start(out=stg_b, in_=wbf_sh[:, :])
            nc.sync.dma_start(out=wbf_loc[:, :], in_=stg_b)
            stg_tab = stgp.tile([128, (NLOC // 128) * D_IN], bf16,
                                tag="stg_tab")
            for k in range(NLOC // 128):
                nc.sync.dma_start(out=stg_tab[:, k * D_IN:(k + 1) * D_IN],
                                  in_=tab_shard[k * 128:(k + 1) * 128, :])
            for k in range(NLOC // 128):
                nc.sync.dma_start(out=tab_loc[k * 128:(k + 1) * 128, :],
                                  in_=stg_tab[:, k * D_IN:(k + 1) * D_IN])

        # device-side reassembly of the replicated tensors
        nc.gpsimd.collective_compute(
            "AllGather", mybir.AluOpType.bypass, replica_groups=grp,
            ins=[wf32_loc[:, :]], outs=[wf32[:, :]])
        nc.gpsimd.collective_compute(
            "AllGather", mybir.AluOpType.bypass, replica_groups=grp,
            ins=[wbf_loc[:, :]], outs=[wbf[:, :]])
        for c in range(NLOC // L1G):
            nc.gpsimd.collective_compute(
                "AllGather", mybir.AluOpType.bypass, replica_groups=grp,
                ins=[tab_loc[c * L1G:(c + 1) * L1G, :]],
                outs=[tab_full[c * NCORE * L1G:(c + 1) * NCORE * L1G, :]])

        consts = ctx.enter_context(tc.tile_pool(name="consts", bufs=1))

        def wload(blob, off, shape, dtype, tag, rows=128):
            o, w = off
            assert int(np.prod(shape[1:])) == w and shape[0] == rows
            t = consts.tile(shape, dtype, tag=tag)
            nc.sync.dma_start(out=t, in_=blob[0:rows, o:o + w])
            return t

        # replicate gather indices to the 8 gpsimd cores' partition stripes
        idxs_sb = consts.tile([128, DEG + 1, NLOC // 16], i16, tag="idxs")
        for k in range(8):
            nc.sync.dma_start(out=idxs_sb[16 * k:16 * (k + 1), :, :],
                              in_=idxs_all[:, :, :])


        wihT1_sb = wload(wbf, BOFF["wihT1"], [128, 4 * D_IN], bf16, "wihT1")
        whhT1_sb = wload(wf32, FOFF["whhT1"], [128, 4 * D_IN], f32, "whhT1")
        blstm1_sb = wload(wf32, FOFF["blstm1"], [128, 4], f32, "blstm1")
        wself1_sb = wload(wbf, BOFF["wself1"], [128, D_FEAT], bf16, "wself1")
        wneigh1_sb = wload(wf32, FOFF["wneigh1"], [128, D_FEAT], f32, "wneigh1")
        b1bc_sb = wload(wf32, FOFF["b1bc"], [128, D_FEAT], f32, "b1bc")
        wihT2_sb = wload(wbf, BOFF["wihT2"], [128, 2 * 4 * D_FEAT], bf16, "wihT2")
        whhT2_sb = wload(wf32, FOFF["whhT2"], [128, 2 * 4 * D_FEAT], f32, "whhT2")
        blstm2_sb = wload(wf32, FOFF["blstm2"], [128, 8], f32, "blstm2")
        wself2_sb = wload(wbf, BOFF["wself2"], [128, 2 * D_FEAT], bf16, "wself2")
        wneigh2_sb = wload(wf32, FOFF["wneigh2"], [128, 2 * D_FEAT], f32, "wneigh2")
        b2bc_sb = wload(wf32, FOFF["b2bc"], [128, D_FEAT], f32, "b2bc")
        wmu_sb = wload(wf32, FOFF["wmu"], [128, 2 * D_REP], f32, "wmu")
        bmu_sb = wload(wf32, FOFF["bmu"], [G, D_REP], f32, "bmu", rows=G)
        wsig_sb = wload(wf32, FOFF["wsig"], [128, 2 * D_REP], f32, "wsig")
        bsig_sb = wload(wf32, FOFF["bsig"], [G, D_REP], f32, "bsig", rows=G)
        iota_sb = wload(wf32, FOFF["iota"], [128, G], f32, "iota")
        pm_sb = consts.tile([128, 2 * NB], f32, tag="poolmeta")
        nc.sync.dma_start(out=pm_sb, in_=poolmeta[:, :])

        # build the one-hot/scaled pooling matrix on device:
        # poolA[p, blk, g] = (g == gid[p, blk]) * inv[p, blk]
        poolA_sb = consts.tile([128, NB, G], f32, tag="poolA")
        for blk in range(NB):
            nc.vector.tensor_scalar(
                poolA_sb[:, blk, :], iota_sb,
                scalar1=pm_sb[:, blk:blk + 1],
                scalar2=pm_sb[:, NB + blk:NB + blk + 1],
                op0=mybir.AluOpType.is_equal, op1=mybir.AluOpType.mult)

        gts = ctx.enter_context(tc.tile_pool(name="gts", bufs=2))
        xgp = ctx.enter_context(tc.tile_pool(name="xgp", bufs=4))
        snp = ctx.enter_context(tc.tile_pool(name="snp", bufs=3))

        GATES = [("i", Sig), ("f", Sig), ("g", Tnh), ("o", Sig)]

        # ================= Layer 1 =================
        # Per node-group: LSTM -> self/neigh -> AllGather of that chunk, so
        # each chunk's collective overlaps the next group's LSTM compute.
        # h1_full is chunk-major ([chunk][rank][j]); the host permutes every
        # gather index to match (tab_full gets the same layout for free from
        # the chunked AllGathers above).
        with tc.tile_pool(name="st1", bufs=1) as st1:
            hN1 = st1.tile([128, NLOC], f32, tag="hN1")
            cN1 = st1.tile([128, NLOC], f32, tag="cN1")
            nc.vector.memset(hN1, 0.0)
            nc.vector.memset(cN1, 0.0)
            featT = st1.tile([128, 1, NLOC], bf16, tag="featT")
            nc.gpsimd.dma_gather(featT[:], tab_shard[:], idxs_sb[:, DEG, :],
                                 NLOC, NLOC, D_IN, transpose=True,
                                 single_packet=False)

            with tc.tile_pool(name="psl1", bufs=3, space="PSUM") as psl, \
                 tc.tile_pool(name="psm1", bufs=2, space="PSUM") as psm:
                for g in range(NLOC // L1G):
                    gsl = slice(g * L1G, (g + 1) * L1G)
                    isl = slice(g * (L1G // 16), (g + 1) * (L1G // 16))
                    for t in range(DEG):
                        xg = xgp.tile([128, 1, L1G], bf16, tag="xg1")
                        nc.gpsimd.dma_gather(
                            xg[:], tab_full[:], idxs_sb[:, t, isl],
                            L1G, L1G, D_IN, transpose=True,
                            single_packet=False)
                        gate_sb = {}
                        for gi, (gn, func) in enumerate(GATES):
                            ps = psl.tile([128, L1G], f32, tag="ps1")
                            wsl = slice(gi * 128, (gi + 1) * 128)
                            for nh in range(L1G // 512):
                                o = ps[:, nh * 512:(nh + 1) * 512]
                                nc.tensor.matmul(
                                    o, wihT1_sb[:, wsl],
                                    xg[:, 0, nh * 512:(nh + 1) * 512],
                                    start=True, stop=False)
                                nc.tensor.matmul(
                                    o, whhT1_sb[:, wsl],
                                    hN1[:, g * L1G + nh * 512:g * L1G + (nh + 1) * 512],
                                    start=False, stop=True)
                            gt = gts.tile([128, L1G], f32, tag=f"gt{gn}")
                            nc.scalar.activation(gt, ps[:, :], func,
                                                 bias=blstm1_sb[:, gi:gi + 1])
                            gate_sb[gn] = gt
                        t0 = gts.tile([128, L1G], f32, tag="t0")
                        nc.vector.tensor_mul(t0, gate_sb["i"], gate_sb["g"])
                        nc.vector.tensor_mul(cN1[:, gsl], cN1[:, gsl], gate_sb["f"])
                        nc.vector.tensor_add(cN1[:, gsl], cN1[:, gsl], t0)
                        tch = gts.tile([128, L1G], f32, tag="tch")
                        nc.scalar.activation(tch, cN1[:, gsl], Tnh)
                        nc.vector.tensor_mul(hN1[:, gsl], gate_sb["o"], tch)

                    # self/neigh + relu for this group's blocks -> h1_shard
                    for blk in range(g * (L1G // 128), (g + 1) * (L1G // 128)):
                        bsl = slice(blk * 128, (blk + 1) * 128)
                        ps = psm.tile([128, D_FEAT], f32, tag="psm1")
                        nc.tensor.matmul(ps, featT[:, 0, bsl], wself1_sb[:, :],
                                         start=True, stop=False)
                        nc.tensor.matmul(ps, hN1[:, bsl], wneigh1_sb[:, :],
                                         start=False, stop=True)
                        tmp = snp.tile([128, D_FEAT], f32, tag="sn1t")
                        nc.vector.tensor_add(tmp, ps, b1bc_sb)
                        h1b = snp.tile([128, D_FEAT], bf16, tag="sn1b")
                        nc.scalar.activation(h1b, tmp, Rlu)
                        nc.sync.dma_start(out=h1_shard[bsl, :], in_=h1b)
                    # all-gather this chunk; overlaps the next group's LSTM
                    nc.gpsimd.collective_compute(
                        "AllGather", mybir.AluOpType.bypass,
                        replica_groups=grp,
                        ins=[h1_shard[g * L1G:(g + 1) * L1G, :]],
                        outs=[h1_full[g * NCORE * L1G:
                                      (g + 1) * NCORE * L1G, :]])

        import concourse.mybir as _mb

        # ================= Layer 2 =================
        with tc.tile_pool(name="st2", bufs=1) as st2:
            hN2 = st2.tile([128, 2, NLOC], f32, tag="hN2")
            cN2 = st2.tile([128, 2, NLOC], f32, tag="cN2")
            nc.vector.memset(hN2, 0.0)
            nc.vector.memset(cN2, 0.0)

            with tc.tile_pool(name="psl2", bufs=4, space="PSUM") as psl:
                for g in range(NLOC // L2G):
                    gsl = slice(g * L2G, (g + 1) * L2G)
                    isl = slice(g * (L2G // 16), (g + 1) * (L2G // 16))
                    for t in range(DEG):
                        xg = xgp.tile([128, 2, L2G], bf16, tag="xg2")
                        nc.gpsimd.dma_gather(
                            xg[:], h1_full[:], idxs_sb[:, t, isl],
                            L2G, L2G, D_FEAT, transpose=True,
                            single_packet=False)
                        gate_sb = {}
                        for gi, (gn, func) in enumerate(GATES):
                            ps = psl.tile([128, 2, L2G], f32, tag="ps2")
                            gt = gts.tile([128, 2, L2G], f32, tag=f"gt{gn}")
                            for mb in range(2):
                                o = ps[:, mb, :]
                                ws = gi * 256 + mb * 128
                                for kb in range(2):
                                    nc.tensor.matmul(
                                        o,
                                        wihT2_sb[:, kb * 1024 + ws:
                                                 kb * 1024 + ws + 128],
                                        xg[:, kb, :],
                                        start=(kb == 0), stop=False)
                                for kb in range(2):
                                    nc.tensor.matmul(
                                        o,
                                        whhT2_sb[:, kb * 1024 + ws:
                                                 kb * 1024 + ws + 128],
                                        hN2[:, kb, gsl],
                                        start=False, stop=(kb == 1))
                                nc.scalar.activation(
                                    gt[:, mb, :], o, func,
                                    bias=blstm2_sb[:, 2 * gi + mb:2 * gi + mb + 1])
                            gate_sb[gn] = gt
                        t0 = gts.tile([128, 2, L2G], f32, tag="t0")
                        nc.vector.tensor_mul(t0, gate_sb["i"], gate_sb["g"])
                        nc.vector.tensor_mul(cN2[:, :, gsl], cN2[:, :, gsl],
                                             gate_sb["f"])
                        nc.vector.tensor_add(cN2[:, :, gsl], cN2[:, :, gsl], t0)
                        tch = gts.tile([128, 2, L2G], f32, tag="tch")
                        nc.scalar.activation(tch, cN2[:, :, gsl], Tnh)
                        nc.vector.tensor_mul(hN2[:, :, gsl], gate_sb["o"], tch)

            # L2 self/neigh + pooling
            h1T = st2.tile([128, 2, NLOC], bf16, tag="h1T")
            nc.gpsimd.dma_gather(h1T[:], h1_shard[:], idxs_sb[:, DEG, :],
                                 NLOC, NLOC, D_FEAT, transpose=True,
                                 single_packet=False)
            with tc.tile_pool(name="psm2", bufs=2, space="PSUM") as psm, \
                 tc.tile_pool(name="pspool", bufs=2, space="PSUM") as psp, \
                 tc.tile_pool(name="pshead", bufs=2, space="PSUM") as psh:
                pool_ps = [psp.tile([128, G], f32, tag=f"pool{mh}",
                                    name=f"pool_ps{mh}")
                           for mh in range(2)]
                for blk in range(NB):
                    bsl = slice(blk * 128, (blk + 1) * 128)
                    ps = psm.tile([128, D_FEAT], f32, tag="psm2")
                    for kb in range(2):
                        nc.tensor.matmul(ps, h1T[:, kb, bsl],
                                         wself2_sb[:, kb * 256:(kb + 1) * 256],
                                         start=(kb == 0), stop=False)
                    for kb in range(2):
                        nc.tensor.matmul(ps, hN2[:, kb, bsl],
                                         wneigh2_sb[:, kb * 256:(kb + 1) * 256],
                                         start=False, stop=(kb == 1))
                    h2sb = snp.tile([128, D_FEAT], f32, tag="h2sb")
                    nc.vector.tensor_add(h2sb, ps, b2bc_sb)
                    for mh in range(2):
                        nc.tensor.matmul(
                            pool_ps[mh], h2sb[:, mh * 128:(mh + 1) * 128],
                            poolA_sb[:, blk, :],
                            start=(blk == 0), stop=(blk == NB - 1),
                            skip_group_check=True)
                prcp = snp.tile([128, 2, G], f32, tag="prcp")
                for mh in range(2):
                    nc.vector.tensor_copy(prcp[:, mh, :], pool_ps[mh])
                nc.sync.dma_start(out=pr_in[:, :, :], in_=prcp)
                nc.gpsimd.collective_compute(
                    "AllReduce", _mb.AluOpType.add,
                    replica_groups=grp,
                    ins=[pr_in[:]], outs=[pr_out[:]])
                prx = snp.tile([128, 2, G], f32, tag="prx")
                nc.sync.dma_start(out=prx, in_=pr_out[:, :, :])
                for wsb, bsb, osb in ((wmu_sb, bmu_sb, out_mu),
                                      (wsig_sb, bsig_sb, out_sigma)):
                    ph = psh.tile([G, D_REP], f32, tag="ph")
                    for kb in range(2):
                        nc.tensor.matmul(ph, prx[:, kb, :],
                                         wsb[:, kb * D_REP:(kb + 1) * D_REP],
                                         start=(kb == 0), stop=(kb == 1))
                    ores = snp.tile([G, D_REP], f32, tag="ores")
                    nc.vector.tensor_add(ores, ph, bsb)
                    nc.sync.dma_start(out=osb[:, :], in_=ores)

    nc.compile()
    return nc


def make_inmaps(inputs, N, DEG, G, NCORE):
    """Host-side preprocessing: shard + reformat the full inputs per core."""
    NLOC = N // NCORE
    NB = NLOC // 128
    FOFF, F32C = _f32_layout()
    BOFF, BFC = _bf_layout()

    feat = np.asarray(inputs["in_feat"], dtype=F32)
    nbr = np.asarray(inputs["neighbors"], dtype=np.int64)
    n2g = np.asarray(inputs["node2graph"], dtype=np.int64)

    def A(name):
        return np.asarray(inputs[name], dtype=F32)

    # chunk-major row permutation matching the on-device chunked AllGather:
    # node (rank r, chunk c, offset j) lives at table row c*(NCORE*L1G)+r*L1G+j
    L1G = 1024 if NLOC % 1024 == 0 else 512
    nodes = np.arange(N)
    r_, rem = nodes // NLOC, nodes % NLOC
    P = (rem // L1G) * (NCORE * L1G) + r_ * L1G + (rem % L1G)
    nbrP = P[nbr]

    def wrap_idx(ids):
        # ids [n] -> [16, n//16] int16 (wrapped in 16 partitions; the device
        # replicates to the 8 gpsimd cores' partition stripes).
        n = ids.shape[0]
        return ids.reshape(n // 16, 16).T.astype(np.int16)

    # ---- packed weight blobs (partition-sharded upload) ----
    wf32 = np.zeros((128, F32C), F32)

    def put32(tag, arr, rows=128):
        o, w = FOFF[tag]
        assert arr.shape == (rows, w), (tag, arr.shape, rows, w)
        wf32[0:rows, o:o + w] = arr

    put32("whhT1", np.ascontiguousarray(A("w_hh1").T))
    put32("wneigh1", A("w_neigh1"))
    put32("b1bc", np.tile(A("b1")[None, :], (128, 1)))
    put32("whhT2", np.ascontiguousarray(
        A("w_hh2").T.reshape(2, 128, 4 * D_FEAT).transpose(1, 0, 2)).reshape(128, -1))
    put32("wneigh2", np.ascontiguousarray(
        A("w_neigh2").reshape(2, 128, D_FEAT).transpose(1, 0, 2)).reshape(128, -1))
    put32("b2bc", np.tile(A("b2")[None, :], (128, 1)))
    put32("wmu", np.ascontiguousarray(
        A("w_mu").reshape(2, 128, D_REP).transpose(1, 0, 2)).reshape(128, -1))
    put32("wsig", np.ascontiguousarray(
        A("w_sigma").reshape(2, 128, D_REP).transpose(1, 0, 2)).reshape(128, -1))
    put32("blstm1", np.ascontiguousarray(A("b_lstm1").reshape(4, 128).T))
    put32("blstm2", np.ascontiguousarray(
        A("b_lstm2").reshape(4, 2, 128).transpose(2, 0, 1).reshape(128, 8)))
    put32("iota", np.tile(np.arange(G, dtype=F32)[None, :], (128, 1)))
    put32("bmu", np.tile(A("b_mu")[None, :], (G, 1)), rows=G)
    put32("bsig", np.tile(A("b_sigma")[None, :], (G, 1)), rows=G)

    wbf = np.zeros((128, BFC), BF)

    def putbf(tag, arr):
        o, w = BOFF[tag]
        assert arr.shape == (128, w), (tag, arr.shape, w)
        wbf[:, o:o + w] = arr.astype(BF)

    putbf("wihT1", np.ascontiguousarray(A("w_ih1").T))
    putbf("wself1", A("w_self1"))
    putbf("wihT2", np.ascontiguousarray(
        A("w_ih2").T.reshape(2, 128, 4 * D_FEAT).transpose(1, 0, 2)).reshape(128, -1))
    putbf("wself2", np.ascontiguousarray(
        A("w_self2").reshape(2, 128, D_FEAT).transpose(1, 0, 2)).reshape(128, -1))

    cnt = np.bincount(n2g, minlength=G).astype(F32)
    inv = 1.0 / np.maximum(cnt, 1.0)
    arange_w = wrap_idx(np.arange(NLOC))  # [16, NLOC//16]

    featBF = feat.astype(BF)
    RS = 128 // NCORE
    in_maps = []
    for c in range(NCORE):
        base = c * NLOC
        # slots 0..DEG-1: neighbor indices (chunk-major); slot DEG: arange
        idxs_all = np.empty((16, DEG + 1, NLOC // 16), np.int16)
        for t in range(DEG):
            idxs_all[:, t, :] = wrap_idx(nbrP[base:base + NLOC, t])
        idxs_all[:, DEG, :] = arange_w
        gl = n2g[base:base + NLOC].reshape(NB, 128)  # [blk, j]
        pm = np.empty((128, 2 * NB), F32)
        pm[:, :NB] = gl.T.astype(F32)
        pm[:, NB:] = inv[gl].T
        m = dict(
            tab_shard=featBF[base:base + NLOC],
            idxs_all=idxs_all,
            poolmeta=pm,
            wf32_sh=np.ascontiguousarray(wf32[c * RS:(c + 1) * RS]),
            wbf_sh=np.ascontiguousarray(wbf[c * RS:(c + 1) * RS]),
        )
        in_maps.append(m)
    return in_maps


_PROG = None


def kernel(**inputs):
    global _PROG
    from concourse.bass_utils import run_bass_kernel_spmd

    cfg = FULL
    if _PROG is None:
        _PROG = build_program(**cfg)
    in_maps = make_inmaps(inputs, **cfg)
    res = run_bass_kernel_spmd(_PROG, in_maps, core_ids=list(range(cfg["NCORE"])))
    mu = np.asarray(res.results[0]["out_mu"], dtype=np.float32)
    sigma = np.asarray(res.results[0]["out_sigma"], dtype=np.float32)
    return (mu, sigma)


# revision 11
# speedup vs baseline: 27.1453x; 3.8961x over previous
"""Trainium2 Bass kernel for a 2-layer GraphSAGE (LSTM aggregator) GNN encoder.

Math (matches the fp32 jax reference):
  L1: h1 = relu(feat @ Wself1 + LSTM16(feat[nbr]) @ Wneigh1 + b1)
  L2: h2 = h1 @ Wself2 + LSTM16(h1[nbr]) @ Wneigh2 + b2
  pool: x[g] = mean_{node in graph g} h2 ; heads: (x@Wmu+bmu, x@Wsig+bsig)

Distribution: nodes sharded across 8 cores (4096 each). The host->device
link (axon tunnel) is very slow, so per-call upload is minimized:
  - feature table: each core uploads only its 1MB shard; the full
    (chunk-major) table is reassembled on device with chunked AllGathers.
  - weights: packed into one f32 and one bf16 blob, uploaded as 1/8
    partition-shards and AllGathered on device.
  - gather indices: uploaded once per core at 1/8 partition height and
    replicated to 128 partitions on device.
  - pooling matrix: built on device from per-node graph ids + inverse
    counts with a fused is_equal/mult tensor_scalar against an iota row.

On-core layout: the LSTM runs feature-major (gates^T = W @ X^T), with the
gathered neighbor features delivered directly in feature-major layout by
dma_gather(transpose=True) from bf16 tables in DRAM. LSTM state h/c stays
fp32; the ih-term matmuls are bf16 (inputs are bf16-rounded activations), the
hh-term matmuls are fp32. Per-graph sums are computed per-core against global
graph ids and all-reduced; head matmuls run redundantly on every core.
"""

import numpy as np
import ml_dtypes

# persistent XLA compilation cache: without it every run_bass_kernel_spmd
# call re-runs the full BIR->NEFF (walrus) compile, ~1s per call.
try:
    import jax
    jax.config.update("jax_compilation_cache_dir", "/tmp/jax_cache")
    jax.config.update("jax_persistent_cache_min_compile_time_secs", 0)
    jax.config.update("jax_persistent_cache_min_entry_size_bytes", 0)
except Exception:
    pass

BF = ml_dtypes.bfloat16
F32 = np.float32

# full problem config
FULL = dict(N=32768, DEG=16, G=64, NCORE=8)
D_IN, D_FEAT, D_REP = 128, 256, 128


def _f32_layout():
    segs = [("whhT1", 512), ("wneigh1", 256), ("b1bc", 256), ("whhT2", 2048),
            ("wneigh2", 512), ("b2bc", 256), ("wmu", 256), ("wsig", 256),
            ("blstm1", 4), ("blstm2", 8), ("iota", 64), ("bmu", 128),
            ("bsig", 128)]
    off, o = {}, 0
    for n, w in segs:
        off[n] = (o, w)
        o += w
    o = (o + 15) // 16 * 16
    return off, o


def _bf_layout():
    segs = [("wihT1", 512), ("wself1", 256), ("wihT2", 2048), ("wself2", 512)]
    off, o = {}, 0
    for n, w in segs:
        off[n] = (o, w)
        o += w
    o = (o + 15) // 16 * 16
    return off, o


def build_program(N, DEG, G, NCORE, stop_after="full"):
    """Build + compile the SPMD Bass program. Returns the Bacc object."""
    from contextlib import ExitStack

    import concourse.mybir as mybir
    import concourse.tile as tile
    from concourse import bacc, library_config

    f32 = mybir.dt.float32
    bf16 = mybir.dt.bfloat16
    i16 = mybir.dt.int16
    Sig = mybir.ActivationFunctionType.Sigmoid
    Tnh = mybir.ActivationFunctionType.Tanh
    Rlu = mybir.ActivationFunctionType.Relu

    NLOC = N // NCORE
    assert NLOC % 128 == 0
    L1G = 1024 if NLOC % 1024 == 0 else 512  # L1 node-group size
    L2G = 512                                # L2 node-group size
    NB = NLOC // 128                         # 128-node blocks
    shared = "Shared" if NCORE > 4 else "Local"
    grp = [list(range(NCORE))]

    FOFF, F32C = _f32_layout()
    BOFF, BFC = _bf_layout()

    nc = bacc.Bacc("TRN2", target_bir_lowering=False, debug=False,
                   num_devices=NCORE)

    # ---- DRAM I/O (per-core, minimized for the slow host link) ----
    tab_shard = nc.dram_tensor("tab_shard", [NLOC, D_IN], bf16,
                               kind="ExternalInput")
    # [16, DEG+1, NLOC//16]: slots 0..DEG-1 = neighbor gather indices into the
    # chunk-major full table; slot DEG = local arange (for featT/h1T gathers).
    idxs_all = nc.dram_tensor("idxs_all", [16, DEG + 1, NLOC // 16], i16,
                              kind="ExternalInput")
    # [128, 2*NB]: cols 0:NB per-node graph id, NB:2*NB inverse graph size
    poolmeta = nc.dram_tensor("poolmeta", [128, 2 * NB], f32,
                              kind="ExternalInput")
    wf32_sh = nc.dram_tensor("wf32_sh", [128 // NCORE, F32C], f32,
                             kind="ExternalInput")
    wbf_sh = nc.dram_tensor("wbf_sh", [128 // NCORE, BFC], bf16,
                            kind="ExternalInput")

    # single output tensor: [0]=mu, [1]=sigma (fewer per-shard fetch
    # round-trips through the slow host link)
    out_cat = nc.dram_tensor("out_cat", [2, G, D_REP], f32,
                             kind="ExternalOutput")

    # ---- Internal DRAM ----
    # collectives may not read ExternalInput tensors; stage through these
    tab_loc = nc.dram_tensor("tab_loc", [NLOC, D_IN], bf16, kind="Internal")
    wf32_loc = nc.dram_tensor("wf32_loc", [128 // NCORE, F32C], f32,
                              kind="Internal")
    wbf_loc = nc.dram_tensor("wbf_loc", [128 // NCORE, BFC], bf16,
                             kind="Internal")
    tab_full = nc.dram_tensor("tab_full", [N, D_IN], bf16, kind="Internal",
                              addr_space=shared)
    wf32 = nc.dram_tensor("wf32", [128, F32C], f32, kind="Internal",
                          addr_space=shared)
    wbf = nc.dram_tensor("wbf", [128, BFC], bf16, kind="Internal",
                         addr_space=shared)
    h1_shard = nc.dram_tensor("h1_shard", [NLOC, D_FEAT], bf16, kind="Internal")
    h1_full = nc.dram_tensor("h1_full", [N, D_FEAT], bf16, kind="Internal",
                             addr_space=shared)
    pr_in = nc.dram_tensor("pr_in", [128, 2, G], f32, kind="Internal")
    pr_out = nc.dram_tensor("pr_out", [128, 2, G], f32, kind="Internal",
                            addr_space=shared)

    nc.gpsimd.load_library(library_config.mlp)

    with tile.TileContext(nc) as tc, ExitStack() as ctx:
        # stage ExternalInputs into Internal DRAM via SBUF (collectives may
        # not read IO tensors directly)
        with tc.tile_pool(name="stage", bufs=1) as stgp:
            stg_f = stgp.tile([128 // NCORE, F32C], f32, tag="stg_f")
            nc.sync.dma_start(out=stg_f, in_=wf32_sh[:, :])
            nc.sync.dma_start(out=wf32_loc[:, :], in_=stg_f)
            stg_b = stgp.tile([128 // NCORE, BFC], bf16, tag="stg_b")
            nc.sync.dma_start(out=stg_b, in_=wbf_sh[:, :])
            nc.sync.dma_start(out=wbf_loc[:, :], in_=stg_b)
            stg_tab = stgp.tile([128, (NLOC // 128) * D_IN], bf16,
                                tag="stg_tab")
            for k in range(NLOC // 128):
                nc.sync.dma_start(out=stg_tab[:, k * D_IN:(k + 1) * D_IN],
                                  in_=tab_shard[k * 128:(k + 1) * 128, :])
            for k in range(NLOC // 128):
                nc.sync.dma_start(out=tab_loc[k * 128:(k + 1) * 128, :],
                                  in_=stg_tab[:, k * D_IN:(k + 1) * D_IN])

        # device-side reassembly of the replicated tensors
        nc.gpsimd.collective_compute(
            "AllGather", mybir.AluOpType.bypass, replica_groups=grp,
            ins=[wf32_loc[:, :]], outs=[wf32[:, :]])
        nc.gpsimd.collective_compute(
            "AllGather", mybir.AluOpType.bypass, replica_groups=grp,
            ins=[wbf_loc[:, :]], outs=[wbf[:, :]])
        for c in range(NLOC // L1G):
            nc.gpsimd.collective_compute(
                "AllGather", mybir.AluOpType.bypass, replica_groups=grp,
                ins=[tab_loc[c * L1G:(c + 1) * L1G, :]],
                outs=[tab_full[c * NCORE * L1G:(c + 1) * NCORE * L1G, :]])

        consts = ctx.enter_context(tc.tile_pool(name="consts", bufs=1))

        def wload(blob, off, shape, dtype, tag, rows=128):
            o, w = off
            assert int(np.prod(shape[1:])) == w and shape[0] == rows
            t = consts.tile(shape, dtype, tag=tag)
            nc.sync.dma_start(out=t, in_=blob[0:rows, o:o + w])
            return t

        # replicate gather indices to the 8 gpsimd cores' partition stripes
        idxs_sb = consts.tile([128, DEG + 1, NLOC // 16], i16, tag="idxs")
        for k in range(8):
            nc.sync.dma_start(out=idxs_sb[16 * k:16 * (k + 1), :, :],
                              in_=idxs_all[:, :, :])


        wihT1_sb = wload(wbf, BOFF["wihT1"], [128, 4 * D_IN], bf16, "wihT1")
        whhT1_sb = wload(wf32, FOFF["whhT1"], [128, 4 * D_IN], f32, "whhT1")
        blstm1_sb = wload(wf32, FOFF["blstm1"], [128, 4], f32, "blstm1")
        wself1_sb = wload(wbf, BOFF["wself1"], [128, D_FEAT], bf16, "wself1")
        wneigh1_sb = wload(wf32, FOFF["wneigh1"], [128, D_FEAT], f32, "wneigh1")
        b1bc_sb = wload(wf32, FOFF["b1bc"], [128, D_FEAT], f32, "b1bc")
        wihT2_sb = wload(wbf, BOFF["wihT2"], [128, 2 * 4 * D_FEAT], bf16, "wihT2")
        whhT2_sb = wload(wf32, FOFF["whhT2"], [128, 2 * 4 * D_FEAT], f32, "whhT2")
        blstm2_sb = wload(wf32, FOFF["blstm2"], [128, 8], f32, "blstm2")
        wself2_sb = wload(wbf, BOFF["wself2"], [128, 2 * D_FEAT], bf16, "wself2")
        wneigh2_sb = wload(wf32, FOFF["wneigh2"], [128, 2 * D_FEAT], f32, "wneigh2")
        b2bc_sb = wload(wf32, FOFF["b2bc"], [128, D_FEAT], f32, "b2bc")
        wmu_sb = wload(wf32, FOFF["wmu"], [128, 2 * D_REP], f32, "wmu")
        bmu_sb = wload(wf32, FOFF["bmu"], [G, D_REP], f32, "bmu", rows=G)
        wsig_sb = wload(wf32, FOFF["wsig"], [128, 2 * D_REP], f32, "wsig")
        bsig_sb = wload(wf32, FOFF["bsig"], [G, D_REP], f32, "bsig", rows=G)
        iota_sb = wload(wf32, FOFF["iota"], [128, G], f32, "iota")
        pm_sb = consts.tile([128, 2 * NB], f32, tag="poolmeta")
        nc.sync.dma_start(out=pm_sb, in_=poolmeta[:, :])

        # build the one-hot/scaled pooling matrix on device:
        # poolA[p, blk, g] = (g == gid[p, blk]) * inv[p, blk]
        poolA_sb = consts.tile([128, NB, G], f32, tag="poolA")
        for blk in range(NB):
            nc.vector.tensor_scalar(
                poolA_sb[:, blk, :], iota_sb,
                scalar1=pm_sb[:, blk:blk + 1],
                scalar2=pm_sb[:, NB + blk:NB + blk + 1],
                op0=mybir.AluOpType.is_equal, op1=mybir.AluOpType.mult)

        gts = ctx.enter_context(tc.tile_pool(name="gts", bufs=2))
        xgp = ctx.enter_context(tc.tile_pool(name="xgp", bufs=4))
        snp = ctx.enter_context(tc.tile_pool(name="snp", bufs=3))

        GATES = [("i", Sig), ("f", Sig), ("g", Tnh), ("o", Sig)]

        # ================= Layer 1 =================
        # Per node-group: LSTM -> self/neigh -> AllGather of that chunk, so
        # each chunk's collective overlaps the next group's LSTM compute.
        # h1_full is chunk-major ([chunk][rank][j]); the host permutes every
        # gather index to match (tab_full gets the same layout for free from
        # the chunked AllGathers above).
        with tc.tile_pool(name="st1", bufs=1) as st1:
            hN1 = st1.tile([128, NLOC], f32, tag="hN1")
            cN1 = st1.tile([128, NLOC], f32, tag="cN1")
            nc.vector.memset(hN1, 0.0)
            nc.vector.memset(cN1, 0.0)
            featT = st1.tile([128, 1, NLOC], bf16, tag="featT")
            nc.gpsimd.dma_gather(featT[:], tab_shard[:], idxs_sb[:, DEG, :],
                                 NLOC, NLOC, D_IN, transpose=True,
                                 single_packet=False)

            with tc.tile_pool(name="psl1", bufs=3, space="PSUM") as psl, \
                 tc.tile_pool(name="psm1", bufs=2, space="PSUM") as psm:
                for g in range(NLOC // L1G):
                    gsl = slice(g * L1G, (g + 1) * L1G)
                    isl = slice(g * (L1G // 16), (g + 1) * (L1G // 16))
                    for t in range(DEG):
                        xg = xgp.tile([128, 1, L1G], bf16, tag="xg1")
                        nc.gpsimd.dma_gather(
                            xg[:], tab_full[:], idxs_sb[:, t, isl],
                            L1G, L1G, D_IN, transpose=True,
                            single_packet=False)
                        gate_sb = {}
                        for gi, (gn, func) in enumerate(GATES):
                            ps = psl.tile([128, L1G], f32, tag="ps1")
                            wsl = slice(gi * 128, (gi + 1) * 128)
                            for nh in range(L1G // 512):
                                o = ps[:, nh * 512:(nh + 1) * 512]
                                nc.tensor.matmul(
                                    o, wihT1_sb[:, wsl],
                                    xg[:, 0, nh * 512:(nh + 1) * 512],
                                    start=True, stop=False)
                                nc.tensor.matmul(
                                    o, whhT1_sb[:, wsl],
                                    hN1[:, g * L1G + nh * 512:g * L1G + (nh + 1) * 512],
                                    start=False, stop=True)
                            gt = gts.tile([128, L1G], f32, tag=f"gt{gn}")
                            nc.scalar.activation(gt, ps[:, :], func,
                                                 bias=blstm1_sb[:, gi:gi + 1])
                            gate_sb[gn] = gt
                        t0 = gts.tile([128, L1G], f32, tag="t0")
                        nc.vector.tensor_mul(t0, gate_sb["i"], gate_sb["g"])
                        nc.vector.tensor_mul(cN1[:, gsl], cN1[:, gsl], gate_sb["f"])
                        nc.vector.tensor_add(cN1[:, gsl], cN1[:, gsl], t0)
                        tch = gts.tile([128, L1G], f32, tag="tch")
                        nc.scalar.activation(tch, cN1[:, gsl], Tnh)
                        nc.vector.tensor_mul(hN1[:, gsl], gate_sb["o"], tch)

                    # self/neigh + relu for this group's blocks -> h1_shard
                    for blk in range(g * (L1G // 128), (g + 1) * (L1G // 128)):
                        bsl = slice(blk * 128, (blk + 1) * 128)
                        ps = psm.tile([128, D_FEAT], f32, tag="psm1")
                        nc.tensor.matmul(ps, featT[:, 0, bsl], wself1_sb[:, :],
                                         start=True, stop=False)
                        nc.tensor.matmul(ps, hN1[:, bsl], wneigh1_sb[:, :],
                                         start=False, stop=True)
                        tmp = snp.tile([128, D_FEAT], f32, tag="sn1t")
                        nc.vector.tensor_add(tmp, ps, b1bc_sb)
                        h1b = snp.tile([128, D_FEAT], bf16, tag="sn1b")
                        nc.scalar.activation(h1b, tmp, Rlu)
                        nc.sync.dma_start(out=h1_shard[bsl, :], in_=h1b)
                    # all-gather this chunk; overlaps the next group's LSTM
                    nc.gpsimd.collective_compute(
                        "AllGather", mybir.AluOpType.bypass,
                        replica_groups=grp,
                        ins=[h1_shard[g * L1G:(g + 1) * L1G, :]],
                        outs=[h1_full[g * NCORE * L1G:
                                      (g + 1) * NCORE * L1G, :]])

        import concourse.mybir as _mb

        # ================= Layer 2 =================
        with tc.tile_pool(name="st2", bufs=1) as st2:
            hN2 = st2.tile([128, 2, NLOC], f32, tag="hN2")
            cN2 = st2.tile([128, 2, NLOC], f32, tag="cN2")
            nc.vector.memset(hN2, 0.0)
            nc.vector.memset(cN2, 0.0)

            with tc.tile_pool(name="psl2", bufs=4, space="PSUM") as psl:
                for g in range(NLOC // L2G):
                    gsl = slice(g * L2G, (g + 1) * L2G)
                    isl = slice(g * (L2G // 16), (g + 1) * (L2G // 16))
                    for t in range(DEG):
                        xg = xgp.tile([128, 2, L2G], bf16, tag="xg2")
                        nc.gpsimd.dma_gather(
                            xg[:], h1_full[:], idxs_sb[:, t, isl],
                            L2G, L2G, D_FEAT, transpose=True,
                            single_packet=False)
                        gate_sb = {}
                        for gi, (gn, func) in enumerate(GATES):
                            ps = psl.tile([128, 2, L2G], f32, tag="ps2")
                            gt = gts.tile([128, 2, L2G], f32, tag=f"gt{gn}")
                            for mb in range(2):
                                o = ps[:, mb, :]
                                ws = gi * 256 + mb * 128
                                for kb in range(2):
                                    nc.tensor.matmul(
                                        o,
                                        wihT2_sb[:, kb * 1024 + ws:
                                                 kb * 1024 + ws + 128],
                                        xg[:, kb, :],
                                        start=(kb == 0), stop=False)
                                for kb in range(2):
                                    nc.tensor.matmul(
                                        o,
                                        whhT2_sb[:, kb * 1024 + ws:
                                                 kb * 1024 + ws + 128],
                                        hN2[:, kb, gsl],
                                        start=False, stop=(kb == 1))
                                nc.scalar.activation(
                                    gt[:, mb, :], o, func,
                                    bias=blstm2_sb[:, 2 * gi + mb:2 * gi + mb + 1])
                            gate_sb[gn] = gt
                        t0 = gts.tile([128, 2, L2G], f32, tag="t0")
                        nc.vector.tensor_mul(t0, gate_sb["i"], gate_sb["g"])
                        nc.vector.tensor_mul(cN2[:, :, gsl], cN2[:, :, gsl],
                                             gate_sb["f"])
                        nc.vector.tensor_add(cN2[:, :, gsl], cN2[:, :, gsl], t0)
                        tch = gts.tile([128, 2, L2G], f32, tag="tch")
                        nc.scalar.activation(tch, cN2[:, :, gsl], Tnh)
                        nc.vector.tensor_mul(hN2[:, :, gsl], gate_sb["o"], tch)

            # L2 self/neigh + pooling
            h1T = st2.tile([128, 2, NLOC], bf16, tag="h1T")
            nc.gpsimd.dma_gather(h1T[:], h1_shard[:], idxs_sb[:, DEG, :],
                                 NLOC, NLOC, D_FEAT, transpose=True,
                                 single_packet=False)
            with tc.tile_pool(name="psm2", bufs=2, space="PSUM") as psm, \
                 tc.tile_pool(name="pspool", bufs=2, space="PSUM") as psp, \
                 tc.tile_pool(name="pshead", bufs=2, space="PSUM") as psh:
                pool_ps = [psp.tile([128, G], f32, tag=f"pool{mh}",
                                    name=f"pool_ps{mh}")
                           for mh in range(2)]
                for blk in range(NB):
                    bsl = slice(blk * 128, (blk + 1) * 128)
                    ps = psm.tile([128, D_FEAT], f32, tag="psm2")
                    for kb in range(2):
                        nc.tensor.matmul(ps, h1T[:, kb, bsl],
                                         wself2_sb[:, kb * 256:(kb + 1) * 256],
                                         start=(kb == 0), stop=False)
                    for kb in range(2):
                        nc.tensor.matmul(ps, hN2[:, kb, bsl],
                                         wneigh2_sb[:, kb * 256:(kb + 1) * 256],
                                         start=False, stop=(kb == 1))
                    h2sb = snp.tile([128, D_FEAT], f32, tag="h2sb")
                    nc.vector.tensor_add(h2sb, ps, b2bc_sb)
                    for mh in range(2):
                        nc.tensor.matmul(
                            pool_ps[mh], h2sb[:, mh * 128:(mh + 1) * 128],
                            poolA_sb[:, blk, :],
                            start=(blk == 0), stop=(blk == NB - 1),
                            skip_group_check=True)
                prcp = snp.tile([128, 2, G], f32, tag="prcp")
                for mh in range(2):
                    nc.vector.tensor_copy(prcp[:, mh, :], pool_ps[mh])
                nc.sync.dma_start(out=pr_in[:, :, :], in_=prcp)
                nc.gpsimd.collective_compute(
                    "AllReduce", _mb.AluOpType.add,
                    replica_groups=grp,
                    ins=[pr_in[:]], outs=[pr_out[:]])
                prx = snp.tile([128, 2, G], f32, tag="prx")
                nc.sync.dma_start(out=prx, in_=pr_out[:, :, :])
                for hi, (wsb, bsb) in enumerate(((wmu_sb, bmu_sb),
                                                 (wsig_sb, bsig_sb))):
                    ph = psh.tile([G, D_REP], f32, tag="ph")
                    for kb in range(2):
                        nc.tensor.matmul(ph, prx[:, kb, :],
                                         wsb[:, kb * D_REP:(kb + 1) * D_REP],
                                         start=(kb == 0), stop=(kb == 1))
                    ores = snp.tile([G, D_REP], f32, tag="ores")
                    nc.vector.tensor_add(ores, ph, bsb)
                    nc.sync.dma_start(out=out_cat[hi, :, :], in_=ores)

    nc.compile()
    return nc


def make_inmaps(inputs, N, DEG, G, NCORE):
    """Host-side preprocessing: shard + reformat the full inputs per core."""
    NLOC = N // NCORE
    NB = NLOC // 128
    FOFF, F32C = _f32_layout()
    BOFF, BFC = _bf_layout()

    feat = np.asarray(inputs["in_feat"], dtype=F32)
    nbr = np.asarray(inputs["neighbors"], dtype=np.int64)
    n2g = np.asarray(inputs["node2graph"], dtype=np.int64)

    def A(name):
        return np.asarray(inputs[name], dtype=F32)

    # chunk-major row permutation matching the on-device chunked AllGather:
    # node (rank r, chunk c, offset j) lives at table row c*(NCORE*L1G)+r*L1G+j
    L1G = 1024 if NLOC % 1024 == 0 else 512
    nodes = np.arange(N)
    r_, rem = nodes // NLOC, nodes % NLOC
    P = (rem // L1G) * (NCORE * L1G) + r_ * L1G + (rem % L1G)
    nbrP = P[nbr]

    def wrap_idx(ids):
        # ids [n] -> [16, n//16] int16 (wrapped in 16 partitions; the device
        # replicates to the 8 gpsimd cores' partition stripes).
        n = ids.shape[0]
        return ids.reshape(n // 16, 16).T.astype(np.int16)

    # ---- packed weight blobs (partition-sharded upload) ----
    wf32 = np.zeros((128, F32C), F32)

    def put32(tag, arr, rows=128):
        o, w = FOFF[tag]
        assert arr.shape == (rows, w), (tag, arr.shape, rows, w)
        wf32[0:rows, o:o + w] = arr

    put32("whhT1", np.ascontiguousarray(A("w_hh1").T))
    put32("wneigh1", A("w_neigh1"))
    put32("b1bc", np.tile(A("b1")[None, :], (128, 1)))
    put32("whhT2", np.ascontiguousarray(
        A("w_hh2").T.reshape(2, 128, 4 * D_FEAT).transpose(1, 0, 2)).reshape(128, -1))
    put32("wneigh2", np.ascontiguousarray(
        A("w_neigh2").reshape(2, 128, D_FEAT).transpose(1, 0, 2)).reshape(128, -1))
    put32("b2bc", np.tile(A("b2")[None, :], (128, 1)))
    put32("wmu", np.ascontiguousarray(
        A("w_mu").reshape(2, 128, D_REP).transpose(1, 0, 2)).reshape(128, -1))
    put32("wsig", np.ascontiguousarray(
        A("w_sigma").reshape(2, 128, D_REP).transpose(1, 0, 2)).reshape(128, -1))
    put32("blstm1", np.ascontiguousarray(A("b_lstm1").reshape(4, 128).T))
    put32("blstm2", np.ascontiguousarray(
        A("b_lstm2").reshape(4, 2, 128).transpose(2, 0, 1).reshape(128, 8)))
    put32("iota", np.tile(np.arange(G, dtype=F32)[None, :], (128, 1)))
    put32("bmu", np.tile(A("b_mu")[None, :], (G, 1)), rows=G)
    put32("bsig", np.tile(A("b_sigma")[None, :], (G, 1)), rows=G)

    wbf = np.zeros((128, BFC), BF)

    def putbf(tag, arr):
        o, w = BOFF[tag]
        assert arr.shape == (128, w), (tag, arr.shape, w)
        wbf[:, o:o + w] = arr.astype(BF)

    putbf("wihT1", np.ascontiguousarray(A("w_ih1").T))
    putbf("wself1", A("w_self1"))
    putbf("wihT2", np.ascontiguousarray(
        A("w_ih2").T.reshape(2, 128, 4 * D_FEAT).transpose(1, 0, 2)).reshape(128, -1))
    putbf("wself2", np.ascontiguousarray(
        A("w_self2").reshape(2, 128, D_FEAT).transpose(1, 0, 2)).reshape(128, -1))

    cnt = np.bincount(n2g, minlength=G).astype(F32)
    inv = 1.0 / np.maximum(cnt, 1.0)
    arange_w = wrap_idx(np.arange(NLOC))  # [16, NLOC//16]

    featBF = feat.astype(BF)
    RS = 128 // NCORE
    in_maps = []
    for c in range(NCORE):
        base = c * NLOC
        # slots 0..DEG-1: neighbor indices (chunk-major); slot DEG: arange
        idxs_all = np.empty((16, DEG + 1, NLOC // 16), np.int16)
        for t in range(DEG):
            idxs_all[:, t, :] = wrap_idx(nbrP[base:base + NLOC, t])
        idxs_all[:, DEG, :] = arange_w
        gl = n2g[base:base + NLOC].reshape(NB, 128)  # [blk, j]
        pm = np.empty((128, 2 * NB), F32)
        pm[:, :NB] = gl.T.astype(F32)
        pm[:, NB:] = inv[gl].T
        m = dict(
            tab_shard=featBF[base:base + NLOC],
            idxs_all=idxs_all,
            poolmeta=pm,
            wf32_sh=np.ascontiguousarray(wf32[c * RS:(c + 1) * RS]),
            wbf_sh=np.ascontiguousarray(wbf[c * RS:(c + 1) * RS]),
        )
        in_maps.append(m)
    return in_maps


_PROG = None


def kernel(**inputs):
    global _PROG
    from concourse.bass_utils import run_bass_kernel_spmd

    cfg = FULL
    if _PROG is None:
        _PROG = build_program(**cfg)
    in_maps = make_inmaps(inputs, **cfg)
    res = run_bass_kernel_spmd(_PROG, in_maps, core_ids=list(range(cfg["NCORE"])))
    oc = np.asarray(res.results[0]["out_cat"], dtype=np.float32)
    return (oc[0], oc[1])


# revision 13
# speedup vs baseline: 34.8922x; 1.2854x over previous
"""Trainium2 Bass kernel for a 2-layer GraphSAGE (LSTM aggregator) GNN encoder.

Math (matches the fp32 jax reference):
  L1: h1 = relu(feat @ Wself1 + LSTM16(feat[nbr]) @ Wneigh1 + b1)
  L2: h2 = h1 @ Wself2 + LSTM16(h1[nbr]) @ Wneigh2 + b2
  pool: x[g] = mean_{node in graph g} h2 ; heads: (x@Wmu+bmu, x@Wsig+bsig)

Distribution: nodes sharded across 8 cores (4096 each). The host->device
link (axon tunnel) is very slow, so per-call upload is minimized:
  - feature table: each core uploads only its 1MB shard; the full
    (chunk-major) table is reassembled on device with chunked AllGathers.
  - weights: packed into one f32 and one bf16 blob, uploaded as 1/8
    partition-shards and AllGathered on device.
  - gather indices: uploaded once per core at 1/8 partition height and
    replicated to 128 partitions on device.
  - pooling matrix: built on device from per-node graph ids + inverse
    counts with a fused is_equal/mult tensor_scalar against an iota row.

On-core layout: the LSTM runs feature-major (gates^T = W @ X^T), with the
gathered neighbor features delivered directly in feature-major layout by
dma_gather(transpose=True) from bf16 tables in DRAM. LSTM state h/c stays
fp32; the ih-term matmuls are bf16 (inputs are bf16-rounded activations), the
hh-term matmuls are fp32. Per-graph sums are computed per-core against global
graph ids and all-reduced; head matmuls run redundantly on every core.
"""

import numpy as np
import ml_dtypes

# persistent XLA compilation cache: without it every run_bass_kernel_spmd
# call re-runs the full BIR->NEFF (walrus) compile, ~1s per call.
try:
    import jax
    jax.config.update("jax_compilation_cache_dir", "/tmp/jax_cache")
    jax.config.update("jax_persistent_cache_min_compile_time_secs", 0)
    jax.config.update("jax_persistent_cache_min_entry_size_bytes", 0)
except Exception:
    pass

BF = ml_dtypes.bfloat16
F32 = np.float32

# full problem config
FULL = dict(N=32768, DEG=16, G=64, NCORE=8)
D_IN, D_FEAT, D_REP = 128, 256, 128


def _f32_layout():
    segs = [("whhT1", 512), ("wneigh1", 256), ("b1bc", 256), ("whhT2", 2048),
            ("wneigh2", 512), ("b2bc", 256), ("wmu", 256), ("wsig", 256),
            ("blstm1", 4), ("blstm2", 8), ("iota", 64), ("bmu", 128),
            ("bsig", 128)]
    off, o = {}, 0
    for n, w in segs:
        off[n] = (o, w)
        o += w
    o = (o + 15) // 16 * 16
    return off, o


def _bf_layout():
    segs = [("wihT1", 512), ("wself1", 256), ("wihT2", 2048), ("wself2", 512)]
    off, o = {}, 0
    for n, w in segs:
        off[n] = (o, w)
        o += w
    o = (o + 15) // 16 * 16
    return off, o


def build_program(N, DEG, G, NCORE, stop_after="full"):
    """Build + compile the SPMD Bass program. Returns the Bacc object."""
    from contextlib import ExitStack

    import concourse.mybir as mybir
    import concourse.tile as tile
    from concourse import bacc, library_config

    f32 = mybir.dt.float32
    bf16 = mybir.dt.bfloat16
    i16 = mybir.dt.int16
    Sig = mybir.ActivationFunctionType.Sigmoid
    Tnh = mybir.ActivationFunctionType.Tanh
    Rlu = mybir.ActivationFunctionType.Relu

    NLOC = N // NCORE
    assert NLOC % 128 == 0
    L1G = 1024 if NLOC % 1024 == 0 else 512  # L1 node-group size
    L2G = 512                                # L2 node-group size
    NB = NLOC // 128                         # 128-node blocks
    shared = "Shared" if NCORE > 4 else "Local"
    grp = [list(range(NCORE))]

    FOFF, F32C = _f32_layout()
    BOFF, BFC = _bf_layout()

    nc = bacc.Bacc("TRN2", target_bir_lowering=False, debug=False,
                   num_devices=NCORE)

    # ---- DRAM I/O (per-core, minimized for the slow host link) ----
    tab_shard = nc.dram_tensor("tab_shard", [NLOC, D_IN], bf16,
                               kind="ExternalInput")
    # [16, DEG+1, NLOC//16]: slots 0..DEG-1 = neighbor gather indices into the
    # chunk-major full table; slot DEG = local arange (for featT/h1T gathers).
    idxs_all = nc.dram_tensor("idxs_all", [16, DEG + 1, NLOC // 16], i16,
                              kind="ExternalInput")
    # [128, 2*NB]: cols 0:NB per-node graph id, NB:2*NB inverse graph size
    poolmeta = nc.dram_tensor("poolmeta", [128, 2 * NB], f32,
                              kind="ExternalInput")
    wf32_sh = nc.dram_tensor("wf32_sh", [128 // NCORE, F32C], f32,
                             kind="ExternalInput")
    wbf_sh = nc.dram_tensor("wbf_sh", [128 // NCORE, BFC], bf16,
                            kind="ExternalInput")

    # single output tensor: [0]=mu, [1]=sigma (fewer per-shard fetch
    # round-trips through the slow host link)
    out_cat = nc.dram_tensor("out_cat", [2, G, D_REP], f32,
                             kind="ExternalOutput")

    # ---- Internal DRAM ----
    # collectives may not read ExternalInput tensors; stage through these
    tab_loc = nc.dram_tensor("tab_loc", [NLOC, D_IN], bf16, kind="Internal")
    wf32_loc = nc.dram_tensor("wf32_loc", [128 // NCORE, F32C], f32,
                              kind="Internal")
    wbf_loc = nc.dram_tensor("wbf_loc", [128 // NCORE, BFC], bf16,
                             kind="Internal")
    tab_full = nc.dram_tensor("tab_full", [N, D_IN], bf16, kind="Internal",
                              addr_space=shared)
    wf32 = nc.dram_tensor("wf32", [128, F32C], f32, kind="Internal",
                          addr_space=shared)
    wbf = nc.dram_tensor("wbf", [128, BFC], bf16, kind="Internal",
                         addr_space=shared)
    h1_shard = nc.dram_tensor("h1_shard", [NLOC, D_FEAT], bf16, kind="Internal")
    h1_full = nc.dram_tensor("h1_full", [N, D_FEAT], bf16, kind="Internal",
                             addr_space=shared)
    pr_in = nc.dram_tensor("pr_in", [128, 2, G], f32, kind="Internal")
    pr_out = nc.dram_tensor("pr_out", [128, 2, G], f32, kind="Internal",
                            addr_space=shared)

    nc.gpsimd.load_library(library_config.mlp)

    with tile.TileContext(nc) as tc, ExitStack() as ctx:
        # stage ExternalInputs into Internal DRAM via SBUF (collectives may
        # not read IO tensors directly)
        with tc.tile_pool(name="stage", bufs=1) as stgp:
            stg_f = stgp.tile([128 // NCORE, F32C], f32, tag="stg_f")
            nc.sync.dma_start(out=stg_f, in_=wf32_sh[:, :])
            nc.sync.dma_start(out=wf32_loc[:, :], in_=stg_f)
            stg_b = stgp.tile([128 // NCORE, BFC], bf16, tag="stg_b")
            nc.sync.dma_start(out=stg_b, in_=wbf_sh[:, :])
            nc.sync.dma_start(out=wbf_loc[:, :], in_=stg_b)
            stg_tab = stgp.tile([128, (NLOC // 128) * D_IN], bf16,
                                tag="stg_tab")
            for k in range(NLOC // 128):
                nc.sync.dma_start(out=stg_tab[:, k * D_IN:(k + 1) * D_IN],
                                  in_=tab_shard[k * 128:(k + 1) * 128, :])
            for k in range(NLOC // 128):
                nc.sync.dma_start(out=tab_loc[k * 128:(k + 1) * 128, :],
                                  in_=stg_tab[:, k * D_IN:(k + 1) * D_IN])

        # device-side reassembly of the replicated tensors
        nc.gpsimd.collective_compute(
            "AllGather", mybir.AluOpType.bypass, replica_groups=grp,
            ins=[wf32_loc[:, :]], outs=[wf32[:, :]])
        nc.gpsimd.collective_compute(
            "AllGather", mybir.AluOpType.bypass, replica_groups=grp,
            ins=[wbf_loc[:, :]], outs=[wbf[:, :]])
        for c in range(NLOC // L1G):
            nc.gpsimd.collective_compute(
                "AllGather", mybir.AluOpType.bypass, replica_groups=grp,
                ins=[tab_loc[c * L1G:(c + 1) * L1G, :]],
                outs=[tab_full[c * NCORE * L1G:(c + 1) * NCORE * L1G, :]])

        consts = ctx.enter_context(tc.tile_pool(name="consts", bufs=1))

        def wload(blob, off, shape, dtype, tag, rows=128):
            o, w = off
            assert int(np.prod(shape[1:])) == w and shape[0] == rows
            t = consts.tile(shape, dtype, tag=tag)
            nc.sync.dma_start(out=t, in_=blob[0:rows, o:o + w])
            return t

        # replicate gather indices to the 8 gpsimd cores' partition stripes
        idxs_sb = consts.tile([128, DEG + 1, NLOC // 16], i16, tag="idxs")
        for k in range(8):
            nc.sync.dma_start(out=idxs_sb[16 * k:16 * (k + 1), :, :],
                              in_=idxs_all[:, :, :])


        wihT1_sb = wload(wbf, BOFF["wihT1"], [128, 4 * D_IN], bf16, "wihT1")
        whhT1_sb = wload(wf32, FOFF["whhT1"], [128, 4 * D_IN], f32, "whhT1")
        blstm1_sb = wload(wf32, FOFF["blstm1"], [128, 4], f32, "blstm1")
        wself1_sb = wload(wbf, BOFF["wself1"], [128, D_FEAT], bf16, "wself1")
        wneigh1_sb = wload(wf32, FOFF["wneigh1"], [128, D_FEAT], f32, "wneigh1")
        b1bc_sb = wload(wf32, FOFF["b1bc"], [128, D_FEAT], f32, "b1bc")
        wihT2_sb = wload(wbf, BOFF["wihT2"], [128, 2 * 4 * D_FEAT], bf16, "wihT2")
        whhT2_sb = wload(wf32, FOFF["whhT2"], [128, 2 * 4 * D_FEAT], f32, "whhT2")
        blstm2_sb = wload(wf32, FOFF["blstm2"], [128, 8], f32, "blstm2")
        wself2_sb = wload(wbf, BOFF["wself2"], [128, 2 * D_FEAT], bf16, "wself2")
        wneigh2_sb = wload(wf32, FOFF["wneigh2"], [128, 2 * D_FEAT], f32, "wneigh2")
        b2bc_sb = wload(wf32, FOFF["b2bc"], [128, D_FEAT], f32, "b2bc")
        wmu_sb = wload(wf32, FOFF["wmu"], [128, 2 * D_REP], f32, "wmu")
        bmu_sb = wload(wf32, FOFF["bmu"], [G, D_REP], f32, "bmu", rows=G)
        wsig_sb = wload(wf32, FOFF["wsig"], [128, 2 * D_REP], f32, "wsig")
        bsig_sb = wload(wf32, FOFF["bsig"], [G, D_REP], f32, "bsig", rows=G)
        iota_sb = wload(wf32, FOFF["iota"], [128, G], f32, "iota")
        pm_sb = consts.tile([128, 2 * NB], f32, tag="poolmeta")
        nc.sync.dma_start(out=pm_sb, in_=poolmeta[:, :])

        # build the one-hot/scaled pooling matrix on device:
        # poolA[p, blk, g] = (g == gid[p, blk]) * inv[p, blk]
        poolA_sb = consts.tile([128, NB, G], f32, tag="poolA")
        for blk in range(NB):
            nc.vector.tensor_scalar(
                poolA_sb[:, blk, :], iota_sb,
                scalar1=pm_sb[:, blk:blk + 1],
                scalar2=pm_sb[:, NB + blk:NB + blk + 1],
                op0=mybir.AluOpType.is_equal, op1=mybir.AluOpType.mult)

        gts = ctx.enter_context(tc.tile_pool(name="gts", bufs=2))
        xgp = ctx.enter_context(tc.tile_pool(name="xgp", bufs=4))
        snp = ctx.enter_context(tc.tile_pool(name="snp", bufs=3))

        GATES = [("i", Sig), ("f", Sig), ("g", Tnh), ("o", Sig)]

        # ================= Layer 1 =================
        # Per node-group: LSTM -> self/neigh -> AllGather of that chunk, so
        # each chunk's collective overlaps the next group's LSTM compute.
        # h1_full is chunk-major ([chunk][rank][j]); the host permutes every
        # gather index to match (tab_full gets the same layout for free from
        # the chunked AllGathers above).
        with tc.tile_pool(name="st1", bufs=1) as st1:
            hN1 = st1.tile([128, NLOC], f32, tag="hN1")
            cN1 = st1.tile([128, NLOC], f32, tag="cN1")
            nc.vector.memset(hN1, 0.0)
            nc.vector.memset(cN1, 0.0)
            featT = st1.tile([128, 1, NLOC], bf16, tag="featT")
            nc.gpsimd.dma_gather(featT[:], tab_shard[:], idxs_sb[:, DEG, :],
                                 NLOC, NLOC, D_IN, transpose=True,
                                 single_packet=False)

            with tc.tile_pool(name="psl1", bufs=3, space="PSUM") as psl, \
                 tc.tile_pool(name="psm1", bufs=2, space="PSUM") as psm:
                for g in range(NLOC // L1G):
                    gsl = slice(g * L1G, (g + 1) * L1G)
                    isl = slice(g * (L1G // 16), (g + 1) * (L1G // 16))

                    def l1_step(t, g=g, gsl=gsl, isl=isl):
                        xg = xgp.tile([128, 1, L1G], bf16, tag="xg1")
                        nc.gpsimd.dma_gather(
                            xg[:], tab_full[:], idxs_sb[:, t, isl],
                            L1G, L1G, D_IN, transpose=True,
                            single_packet=False)
                        gate_sb = {}
                        for gi, (gn, func) in enumerate(GATES):
                            ps = psl.tile([128, L1G], f32, tag="ps1")
                            wsl = slice(gi * 128, (gi + 1) * 128)
                            for nh in range(L1G // 512):
                                o = ps[:, nh * 512:(nh + 1) * 512]
                                nc.tensor.matmul(
                                    o, wihT1_sb[:, wsl],
                                    xg[:, 0, nh * 512:(nh + 1) * 512],
                                    start=True, stop=False)
                                nc.tensor.matmul(
                                    o, whhT1_sb[:, wsl],
                                    hN1[:, g * L1G + nh * 512:g * L1G + (nh + 1) * 512],
                                    start=False, stop=True)
                            gt = gts.tile([128, L1G], f32, tag=f"gt{gn}")
                            nc.scalar.activation(gt, ps[:, :], func,
                                                 bias=blstm1_sb[:, gi:gi + 1])
                            gate_sb[gn] = gt
                        t0 = gts.tile([128, L1G], f32, tag="t0")
                        nc.vector.tensor_mul(t0, gate_sb["i"], gate_sb["g"])
                        nc.vector.tensor_mul(cN1[:, gsl], cN1[:, gsl], gate_sb["f"])
                        nc.vector.tensor_add(cN1[:, gsl], cN1[:, gsl], t0)
                        tch = gts.tile([128, L1G], f32, tag="tch")
                        nc.scalar.activation(tch, cN1[:, gsl], Tnh)
                        nc.vector.tensor_mul(hN1[:, gsl], gate_sb["o"], tch)

                    tc.For_i_unrolled(0, DEG, 1, l1_step, max_unroll=2)

                    # self/neigh + relu for this group's blocks -> h1_shard
                    for blk in range(g * (L1G // 128), (g + 1) * (L1G // 128)):
                        bsl = slice(blk * 128, (blk + 1) * 128)
                        ps = psm.tile([128, D_FEAT], f32, tag="psm1")
                        nc.tensor.matmul(ps, featT[:, 0, bsl], wself1_sb[:, :],
                                         start=True, stop=False)
                        nc.tensor.matmul(ps, hN1[:, bsl], wneigh1_sb[:, :],
                                         start=False, stop=True)
                        tmp = snp.tile([128, D_FEAT], f32, tag="sn1t")
                        nc.vector.tensor_add(tmp, ps, b1bc_sb)
                        h1b = snp.tile([128, D_FEAT], bf16, tag="sn1b")
                        nc.scalar.activation(h1b, tmp, Rlu)
                        nc.sync.dma_start(out=h1_shard[bsl, :], in_=h1b)
                    # all-gather this chunk; overlaps the next group's LSTM
                    nc.gpsimd.collective_compute(
                        "AllGather", mybir.AluOpType.bypass,
                        replica_groups=grp,
                        ins=[h1_shard[g * L1G:(g + 1) * L1G, :]],
                        outs=[h1_full[g * NCORE * L1G:
                                      (g + 1) * NCORE * L1G, :]])

        import concourse.mybir as _mb

        # ================= Layer 2 =================
        with tc.tile_pool(name="st2", bufs=1) as st2:
            hN2 = st2.tile([128, 2, NLOC], f32, tag="hN2")
            cN2 = st2.tile([128, 2, NLOC], f32, tag="cN2")
            nc.vector.memset(hN2, 0.0)
            nc.vector.memset(cN2, 0.0)

            with tc.tile_pool(name="psl2", bufs=4, space="PSUM") as psl:
                for g in range(NLOC // L2G):
                    gsl = slice(g * L2G, (g + 1) * L2G)
                    isl = slice(g * (L2G // 16), (g + 1) * (L2G // 16))

                    def l2_step(t, gsl=gsl, isl=isl):
                        xg = xgp.tile([128, 2, L2G], bf16, tag="xg2")
                        nc.gpsimd.dma_gather(
                            xg[:], h1_full[:], idxs_sb[:, t, isl],
                            L2G, L2G, D_FEAT, transpose=True,
                            single_packet=False)
                        gate_sb = {}
                        for gi, (gn, func) in enumerate(GATES):
                            ps = psl.tile([128, 2, L2G], f32, tag="ps2")
                            gt = gts.tile([128, 2, L2G], f32, tag=f"gt{gn}")
                            for mb in range(2):
                                o = ps[:, mb, :]
                                ws = gi * 256 + mb * 128
                                for kb in range(2):
                                    nc.tensor.matmul(
                                        o,
                                        wihT2_sb[:, kb * 1024 + ws:
                                                 kb * 1024 + ws + 128],
                                        xg[:, kb, :],
                                        start=(kb == 0), stop=False)
                                for kb in range(2):
                                    nc.tensor.matmul(
                                        o,
                                        whhT2_sb[:, kb * 1024 + ws:
                                                 kb * 1024 + ws + 128],
                                        hN2[:, kb, gsl],
                                        start=False, stop=(kb == 1))
                                nc.scalar.activation(
                                    gt[:, mb, :], o, func,
                                    bias=blstm2_sb[:, 2 * gi + mb:2 * gi + mb + 1])
                            gate_sb[gn] = gt
                        t0 = gts.tile([128, 2, L2G], f32, tag="t0")
                        nc.vector.tensor_mul(t0, gate_sb["i"], gate_sb["g"])
                        nc.vector.tensor_mul(cN2[:, :, gsl], cN2[:, :, gsl],
                                             gate_sb["f"])
                        nc.vector.tensor_add(cN2[:, :, gsl], cN2[:, :, gsl], t0)
                        tch = gts.tile([128, 2, L2G], f32, tag="tch")
                        nc.scalar.activation(tch, cN2[:, :, gsl], Tnh)
                        nc.vector.tensor_mul(hN2[:, :, gsl], gate_sb["o"], tch)

                    tc.For_i_unrolled(0, DEG, 1, l2_step, max_unroll=2)

            # L2 self/neigh + pooling
            h1T = st2.tile([128, 2, NLOC], bf16, tag="h1T")
            nc.gpsimd.dma_gather(h1T[:], h1_shard[:], idxs_sb[:, DEG, :],
                                 NLOC, NLOC, D_FEAT, transpose=True,
                                 single_packet=False)
            with tc.tile_pool(name="psm2", bufs=2, space="PSUM") as psm, \
                 tc.tile_pool(name="pspool", bufs=2, space="PSUM") as psp, \
                 tc.tile_pool(name="pshead", bufs=2, space="PSUM") as psh:
                pool_ps = [psp.tile([128, G], f32, tag=f"pool{mh}",
                                    name=f"pool_ps{mh}")
                           for mh in range(2)]
                for blk in range(NB):
                    bsl = slice(blk * 128, (blk + 1) * 128)
                    ps = psm.tile([128, D_FEAT], f32, tag="psm2")
                    for kb in range(2):
                        nc.tensor.matmul(ps, h1T[:, kb, bsl],
                                         wself2_sb[:, kb * 256:(kb + 1) * 256],
                                         start=(kb == 0), stop=False)
                    for kb in range(2):
                        nc.tensor.matmul(ps, hN2[:, kb, bsl],
                                         wneigh2_sb[:, kb * 256:(kb + 1) * 256],
                                         start=False, stop=(kb == 1))
                    h2sb = snp.tile([128, D_FEAT], f32, tag="h2sb")
                    nc.vector.tensor_add(h2sb, ps, b2bc_sb)
                    for mh in range(2):
                        nc.tensor.matmul(
                            pool_ps[mh], h2sb[:, mh * 128:(mh + 1) * 128],
                            poolA_sb[:, blk, :],
                            start=(blk == 0), stop=(blk == NB - 1),
                            skip_group_check=True)
                prcp = snp.tile([128, 2, G], f32, tag="prcp")
                for mh in range(2):
                    nc.vector.tensor_copy(prcp[:, mh, :], pool_ps[mh])
                nc.sync.dma_start(out=pr_in[:, :, :], in_=prcp)
                nc.gpsimd.collective_compute(
                    "AllReduce", _mb.AluOpType.add,
                    replica_groups=grp,
                    ins=[pr_in[:]], outs=[pr_out[:]])
                prx = snp.tile([128, 2, G], f32, tag="prx")
                nc.sync.dma_start(out=prx, in_=pr_out[:, :, :])
                for hi, (wsb, bsb) in enumerate(((wmu_sb, bmu_sb),
                                                 (wsig_sb, bsig_sb))):
                    ph = psh.tile([G, D_REP], f32, tag="ph")
                    for kb in range(2):
                        nc.tensor.matmul(ph, prx[:, kb, :],
                                         wsb[:, kb * D_REP:(kb + 1) * D_REP],
                                         start=(kb == 0), stop=(kb == 1))
                    ores = snp.tile([G, D_REP], f32, tag="ores")
                    nc.vector.tensor_add(ores, ph, bsb)
                    nc.sync.dma_start(out=out_cat[hi, :, :], in_=ores)

    nc.compile()
    return nc


def make_inmaps(inputs, N, DEG, G, NCORE):
    """Host-side preprocessing: shard + reformat the full inputs per core."""
    NLOC = N // NCORE
    NB = NLOC // 128
    FOFF, F32C = _f32_layout()
    BOFF, BFC = _bf_layout()

    feat = np.asarray(inputs["in_feat"], dtype=F32)
    nbr = np.asarray(inputs["neighbors"], dtype=np.int64)
    n2g = np.asarray(inputs["node2graph"], dtype=np.int64)

    def A(name):
        return np.asarray(inputs[name], dtype=F32)

    # chunk-major row permutation matching the on-device chunked AllGather:
    # node (rank r, chunk c, offset j) lives at table row c*(NCORE*L1G)+r*L1G+j
    L1G = 1024 if NLOC % 1024 == 0 else 512
    nodes = np.arange(N)
    r_, rem = nodes // NLOC, nodes % NLOC
    P = (rem // L1G) * (NCORE * L1G) + r_ * L1G + (rem % L1G)
    nbrP = P[nbr]

    def wrap_idx(ids):
        # ids [n] -> [16, n//16] int16 (wrapped in 16 partitions; the device
        # replicates to the 8 gpsimd cores' partition stripes).
        n = ids.shape[0]
        return ids.reshape(n // 16, 16).T.astype(np.int16)

    # ---- packed weight blobs (partition-sharded upload) ----
    wf32 = np.zeros((128, F32C), F32)

    def put32(tag, arr, rows=128):
        o, w = FOFF[tag]
        assert arr.shape == (rows, w), (tag, arr.shape, rows, w)
        wf32[0:rows, o:o + w] = arr

    put32("whhT1", np.ascontiguousarray(A("w_hh1").T))
    put32("wneigh1", A("w_neigh1"))
    put32("b1bc", np.tile(A("b1")[None, :], (128, 1)))
    put32("whhT2", np.ascontiguousarray(
        A("w_hh2").T.reshape(2, 128, 4 * D_FEAT).transpose(1, 0, 2)).reshape(128, -1))
    put32("wneigh2", np.ascontiguousarray(
        A("w_neigh2").reshape(2, 128, D_FEAT).transpose(1, 0, 2)).reshape(128, -1))
    put32("b2bc", np.tile(A("b2")[None, :], (128, 1)))
    put32("wmu", np.ascontiguousarray(
        A("w_mu").reshape(2, 128, D_REP).transpose(1, 0, 2)).reshape(128, -1))
    put32("wsig", np.ascontiguousarray(
        A("w_sigma").reshape(2, 128, D_REP).transpose(1, 0, 2)).reshape(128, -1))
    put32("blstm1", np.ascontiguousarray(A("b_lstm1").reshape(4, 128).T))
    put32("blstm2", np.ascontiguousarray(
        A("b_lstm2").reshape(4, 2, 128).transpose(2, 0, 1).reshape(128, 8)))
    put32("iota", np.tile(np.arange(G, dtype=F32)[None, :], (128, 1)))
    put32("bmu", np.tile(A("b_mu")[None, :], (G, 1)), rows=G)
    put32("bsig", np.tile(A("b_sigma")[None, :], (G, 1)), rows=G)

    wbf = np.zeros((128, BFC), BF)

    def putbf(tag, arr):
        o, w = BOFF[tag]
        assert arr.shape == (128, w), (tag, arr.shape, w)
        wbf[:, o:o + w] = arr.astype(BF)

    putbf("wihT1", np.ascontiguousarray(A("w_ih1").T))
    putbf("wself1", A("w_self1"))
    putbf("wihT2", np.ascontiguousarray(
        A("w_ih2").T.reshape(2, 128, 4 * D_FEAT).transpose(1, 0, 2)).reshape(128, -1))
    putbf("wself2", np.ascontiguousarray(
        A("w_self2").reshape(2, 128, D_FEAT).transpose(1, 0, 2)).reshape(128, -1))

    cnt = np.bincount(n2g, minlength=G).astype(F32)
    inv = 1.0 / np.maximum(cnt, 1.0)
    arange_w = wrap_idx(np.arange(NLOC))  # [16, NLOC//16]

    featBF = feat.astype(BF)
    RS = 128 // NCORE
    in_maps = []
    for c in range(NCORE):
        base = c * NLOC
        # slots 0..DEG-1: neighbor indices (chunk-major); slot DEG: arange
        idxs_all = np.empty((16, DEG + 1, NLOC // 16), np.int16)
        for t in range(DEG):
            idxs_all[:, t, :] = wrap_idx(nbrP[base:base + NLOC, t])
        idxs_all[:, DEG, :] = arange_w
        gl = n2g[base:base + NLOC].reshape(NB, 128)  # [blk, j]
        pm = np.empty((128, 2 * NB), F32)
        pm[:, :NB] = gl.T.astype(F32)
        pm[:, NB:] = inv[gl].T
        m = dict(
            tab_shard=featBF[base:base + NLOC],
            idxs_all=idxs_all,
            poolmeta=pm,
            wf32_sh=np.ascontiguousarray(wf32[c * RS:(c + 1) * RS]),
            wbf_sh=np.ascontiguousarray(wbf[c * RS:(c + 1) * RS]),
        )
        in_maps.append(m)
    return in_maps


_PROG = None


def kernel(**inputs):
    global _PROG
    from concourse.bass_utils import run_bass_kernel_spmd

    cfg = FULL
    if _PROG is None:
        _PROG = build_program(**cfg)
    in_maps = make_inmaps(inputs, **cfg)
    res = run_bass_kernel_spmd(_PROG, in_maps, core_ids=list(range(cfg["NCORE"])))
    oc = np.asarray(res.results[0]["out_cat"], dtype=np.float32)
    return (oc[0], oc[1])
